# revision 43
# baseline (speedup 1.0000x reference)
"""Fused attention block (QKV conv -> 4-head attention -> proj -> BatchNorm -> LeakyReLU)
distributed over 8 trn2 NeuronCores, data-parallel over the batch dim.

Self-contained: hardcodes shapes B=8, C=64, N=2048, H=4.

Per-core layout tricks (v8 — HAM warm-up, fast tail, AllReduce stats):
  - a burst of dummy back-to-back matmuls runs during the input-DMA window
    so the PE's HAM clock gate flips to 8/8 (2.4 GHz) before real work;
  - scores computed transposed (S^T = K^T Q, keys on partitions); the 4 heads'
    score matmuls go to 4 distinct PE row-groups (tile_position=(32h,0));
  - PV matmuls go to 4 distinct col-groups (tile_position=(0,32h)),
    accumulating over key tiles in PSUM;
  - exp is split across engines: heads 0,1 take true exp on the scalar
    engine; heads 2,3 take a Schraudolph fast-exp on the vector engine
    (f32->int16 cast of a*s+b via tensor_scalar, bitcast to fp16), with
    the slope a folded into wk host-side; PV runs two tiles behind the
    scores so the in-order PE queue never waits on either exp path;
  - softmax denominators come free from a ones-column in the PV stationary;
    reciprocal via exp(-ln(x)) on the scalar engine; mid-kernel chunks
    broadcast the reciprocal rows via a DRAM round trip (hidden under
    compute), the LAST chunk broadcasts the raw denominators with one
    matmul against a row-selector matrix and takes the reciprocal of the
    result, keeping the critical tail ~6us shorter;
  - all small params ship in ONE packed DMA (bf16 blob, f32 views bitcast);
  - BatchNorm stats cross-core reduced via two [C,4] AllReduces (about half
    the latency of AllGather+local add): the top chunk-pair hidden
    mid-kernel, the bottom pair at the tail with chunk 2's half shipped
    early; a dummy AllReduce at prologue warms the CC stream and tiny
    gpsimd reads keep the trigger path awake;
  - BN finalize: variance pieces and -gamma*mean on the (idle) DVE in
    parallel with the scalar chain; BN scale/shift + LeakyReLU as a split
    Prelu so the first output DMA overlaps the second half's compute.
"""
import numpy as np
import ml_dtypes

import concourse.bass as bass
import concourse.mybir as mybir
from concourse import bacc, tile
from concourse.bass_utils import run_bass_kernel_spmd

B, C, N, H, D = 8, 64, 2048, 4, 16
C2 = 2 * C           # 128 input channels after concat
NC = 512             # query-dim chunk = one fp32 PSUM bank
NCH = N // NC        # 4 chunks
MT = N // 128        # 16 key tiles of 128
F32 = mybir.dt.float32
BF16 = mybir.dt.bfloat16
F16 = mybir.dt.float16
I16 = mybir.dt.int16
SCALE = float(D) ** -0.5
BN_EPS = 1e-5
LEAK = 0.2
N_CORES = 8
CNT = float(B * N)   # batchnorm population count
A_EXP = 1024.0 * 1.4426950408889634   # 2^10 * log2(e)
B_EXP = 15360.0                       # 15 << 10: fp16 exponent bias
N_WARM = 7                            # dummy MMs to flip the HAM clock gate

Alu = mybir.AluOpType
Act = mybir.ActivationFunctionType

# packed-params column layout (bf16 columns)
PK_WQ = 0            # [128,128] bf16
PK_WK = 128          # [128,128] bf16
PK_WV = 256          # [128,64]  bf16
PK_WP = 320          # [128,64]  bf16
PK_FOLD = 384        # [128,128] f32  -> 256 bf16 cols
PK_G = 640           # [128,1] f32 -> 2 cols
PK_B = 642           # [128,1] f32 -> 2 cols
PK_EPS = 644         # [128,1] f32 -> 2 cols
PK_BC = 646          # [128,128] bf16 denom-broadcast selector
PK_COLS = 774


def build():
    nc = bacc.Bacc("TRN2", target_bir_lowering=False, debug=False,
                   num_devices=N_CORES)
    x_p = nc.declare_dram_parameter("x", [C2, N], BF16, isOutput=False)
    wpk_p = nc.declare_dram_parameter("wpk", [C2, PK_COLS], BF16, isOutput=False)
    out_p = nc.declare_dram_parameter("out", [C, N], F32, isOutput=True)

    with tile.TileContext(nc) as tc:
        with (
            tc.tile_pool(name="sb", bufs=1) as sb,
            tc.tile_pool(name="ps_s", bufs=3, space="PSUM") as ps_s,
            tc.tile_pool(name="ps_pv", bufs=2, space="PSUM") as ps_pv,
            tc.tile_pool(name="pp", bufs=8) as pp,
            tc.tile_pool(name="ep", bufs=2) as ep,
            tc.tile_pool(name="dram", bufs=2, space="DRAM") as dram,
        ):
            # ---- persistent SBUF tiles
            x_sb = sb.tile([C2, N], BF16, tag="x")
            wpk_sb = sb.tile([C2, PK_COLS], BF16, tag="wpk")
            wq_sb = wpk_sb[:, PK_WQ:PK_WQ + 128]
            wk_sb = wpk_sb[:, PK_WK:PK_WK + 128]
            wv_sb = wpk_sb[:, PK_WV:PK_WV + C]
            wp_sb = wpk_sb[:, PK_WP:PK_WP + C]
            fold_sb = wpk_sb[:, PK_FOLD:PK_FOLD + 256].bitcast(F32)
            g_sb = wpk_sb[:, PK_G:PK_G + 2].bitcast(F32)
            b_sb = wpk_sb[:, PK_B:PK_B + 2].bitcast(F32)
            eps_t = wpk_sb[:, PK_EPS:PK_EPS + 2].bitcast(F32)
            bc_sb = wpk_sb[:, PK_BC:PK_BC + 128]
            q_sb = sb.tile([C2, N], BF16, tag="q")    # head h rows 32h..32h+16
            k_sb = sb.tile([C2, N], BF16, tag="k")
            # per key-tile, per head: 32 cols = [16 V^T | 1 ones | 15 zeros]
            vt_sb = sb.tile([C2, MT * 128], F16, tag="vt")
            y_sb = sb.tile([C2, 2 * NC], F32, tag="y")  # proj out, fold layout
            yl_sb = sb.tile([C2, 2 * NC], F32, tag="yl")
            stats = sb.tile([C2, 4], F32, tag="stats")
            gwk = sb.tile([1, 2], F16, tag="gwk")   # gpsimd keep-awake scratch
            dmy_sb = sb.tile([8, 2], F32, tag="dmy")
            scr = sb.tile([C2, NC], BF16, tag="scr")  # HAM warm-up operand

            # ---- HAM warm-up: back-to-back dummy matmuls during the input
            # DMA window. ~4us of sustained PE busy flips the clock gate from
            # 4/8 (1.2 GHz) to 8/8 (2.4 GHz); the small inter-MM gaps of the
            # attention body then keep it warm. Nothing consumes the output.
            nc.gpsimd.memset(scr[:], 0.0)
            for w in range(N_WARM):
                wp_ps = ps_pv.tile([C2, NC], F32, tag="pv")
                nc.tensor.matmul(wp_ps[:], lhsT=scr[:, 0:128], rhs=scr[:])

            # ---- prologue loads: x in 512-col pieces so the first QKV
            # matmul starts after 128KB; all small params in ONE packed DMA
            nc.sync.dma_start(x_sb[:, 0:512], x_p[:, 0:512])
            nc.scalar.dma_start(wpk_sb[:], wpk_p[:])
            nc.sync.dma_start(x_sb[:, 512:1024], x_p[:, 512:1024])
            nc.scalar.dma_start(x_sb[:, 1024:1536], x_p[:, 1024:1536])
            nc.sync.dma_start(x_sb[:, 1536:N], x_p[:, 1536:N])

            # V^T zero fill + ones columns on gpsimd (before it blocks on the
            # warm-up collective)
            nc.gpsimd.memset(vt_sb[:], 0.0)
            ones_ap = vt_sb[:].rearrange(
                "q (p h e) -> q p h e", p=MT, h=H, e=32)[:, :, :, 16:17]
            nc.gpsimd.memset(ones_ap, 1.0)

            # ---- QKV projections. q/k evacuated with head h at rows
            # 32h..32h+16 (stationary has zeros elsewhere).
            for c4 in range(4):
                cs = slice(512 * c4, 512 * (c4 + 1))
                qp = ps_pv.tile([C2, NC], F32, tag="pv")
                nc.tensor.matmul(qp[:], lhsT=wq_sb, rhs=x_sb[:, cs])
                nc.scalar.activation(q_sb[:, cs], qp[:], Act.Copy)
                kp = ps_pv.tile([C2, NC], F32, tag="pv")
                nc.tensor.matmul(kp[:], lhsT=wk_sb, rhs=x_sb[:, cs])
                nc.vector.tensor_copy(k_sb[:, cs], kp[:])

            # warm-up AllReduce: wakes the CC stream early so the real one at
            # the tail skips the cold-start latency.
            dm_in = dram.tile([8, 2], F32, tag="dm_in")
            dm_out = dram.tile([8, 2], F32, tag="dm_out")
            nc.gpsimd.memset(dmy_sb[:], 1.0)
            nc.gpsimd.dma_start(dm_in[:], dmy_sb[:])
            nc.gpsimd.collective_compute(
                "AllReduce", Alu.add,
                replica_groups=[list(range(N_CORES))],
                ins=[dm_in.opt()], outs=[dm_out.opt()])

            # all 16 V^T key tiles in ONE psum allocation
            vp_all = ps_s.tile([C2, MT * C], F32, tag="s")
            for p in range(MT):
                nc.tensor.matmul(vp_all[:, C * p:C * (p + 1)],
                                 lhsT=x_sb[:, 128 * p:128 * (p + 1)],
                                 rhs=wv_sb)
            vt_dst = vt_sb[:].rearrange(
                "q (p h e) -> q p h e", p=MT, h=H, e=32)[:, :, :, 0:16]
            vt_src = vp_all[:].rearrange(
                "q (p h d) -> q p h d", p=MT, h=H, d=D)
            nc.vector.tensor_copy(vt_dst, vt_src)

            def epilogue_pre(c, pv, last=False):
                """Denominator chain for chunk c: ln/exp + partition-broadcast.
                Mid-kernel chunks bounce the reciprocal rows through DRAM (the
                round-trip hides behind compute). The LAST chunk sits on the
                critical tail, so it broadcasts via one matmul against a
                row-selector matrix instead (saves the ~8us DMA round trip)."""
                if last:
                    # evacuate pv, broadcast the (positive) denominator rows
                    # to every partition via the selector matmul, then take
                    # the reciprocal of the broadcast result — NaN-free, and
                    # the mul's operands both end up in SBUF
                    pvs = ep.tile([C2, NC], BF16, tag="pvs")
                    nc.scalar.activation(pvs[:], pv[:], Act.Copy)
                    nc.gpsimd.tensor_copy(gwk[:], pvs[0:1, 0:2])
                    rbc_d = ps_s.tile([C2, 2 * NC], F32, tag="s")
                    nc.tensor.matmul(rbc_d[:, 0:NC], lhsT=bc_sb, rhs=pvs[:])
                    dln2 = ep.tile([C2, NC], F32, tag="dln")
                    nc.scalar.activation(dln2[:], rbc_d[:, 0:NC], Act.Ln)
                    rbc = ep.tile([C2, NC], F32, tag="rbc")
                    nc.scalar.activation(rbc[:], dln2[:], Act.Exp, scale=-1.0)
                    nc.gpsimd.tensor_copy(gwk[:], rbc[0:1, 0:2])
                    return (pvs, rbc)
                dln = ep.tile([C2, NC], F32, tag="dln")
                nc.scalar.activation(dln[:], pv[:], Act.Ln)
                drc = ep.tile([C2, NC], F32, tag="drc")
                nc.scalar.activation(drc[:], dln[:], Act.Exp, scale=-1.0)
                rbc = ep.tile([C2, NC], F32, tag="rbc")
                rec_d = dram.tile([H, NC], F32, tag="rec_d")
                for h in range(H):
                    nc.sync.dma_start(rec_d[h:h + 1, :],
                                      drc[32 * h + 16:32 * h + 17, :])
                for h in range(H):
                    nc.sync.dma_start(
                        rbc[32 * h:32 * h + 32, :],
                        rec_d[h:h + 1, :].partition_broadcast(32))
                return rbc

            def epilogue_post(c, pv, rbc, last=False):
                """Normalize chunk-c attention output, project, evac + stats.
                The proj-output PSUM tile comes from the pv ring (the slot the
                normalize just freed) so the scores ring keeps all 3 slots."""
                on = ep.tile([C2, NC], BF16, tag="on")
                if last:
                    pvs, rbc_sb = rbc
                    nc.vector.tensor_mul(on[:], pvs[:], rbc_sb[:])
                    nc.gpsimd.tensor_copy(gwk[:], on[0:1, 0:2])
                else:
                    nc.vector.tensor_mul(on[:], pv[:], rbc[:])
                yp = ps_s.tile([C2, 2 * NC], F32, tag="s")
                r = slice(64 * (c // 2), 64 * (c // 2) + 64)
                nc.tensor.matmul(yp[r, 0:NC], lhsT=wp_sb, rhs=on[:],
                                 tile_position=(0, 64 * (c // 2)))
                ycols = slice(512 * (c % 2), 512 * (c % 2) + 512)
                s0 = 2 * (c % 2)
                nc.vector.tensor_scalar(y_sb[r, ycols], yp[r, 0:NC], 1.0, 0.0,
                                        op0=Alu.mult, op1=Alu.add,
                                        accum_out=stats[r, s0:s0 + 1])
                ysq = ep.tile([C2, NC], F32, tag="dln")
                if last:
                    # critical tail: sumsq on the scalar engine in parallel
                    # with the DVE evac (both read yp from PSUM)
                    nc.scalar.activation(ysq[r, :], yp[r, 0:NC], Act.Square,
                                         accum_out=stats[r, s0 + 1:s0 + 2])
                else:
                    nc.vector.scalar_tensor_tensor(
                        ysq[r, :], y_sb[r, ycols], 0.0, y_sb[r, ycols],
                        op0=Alu.add, op1=Alu.mult,
                        accum_out=stats[r, s0 + 1:s0 + 2])

            # stats-reduce staging (split: top chunk-pair mid-kernel, bottom
            # pair at the tail). AllReduce on [C,4] — roughly half the CC
            # latency of the AllGather-and-add-locally scheme.
            rb_a = sb.tile([C2, 4], F32, tag="rb_a")
            rb_b = sb.tile([C2, 4], F32, tag="rb_b")
            fa = sb.tile([C2, 2], F32, tag="fa")
            st_in_a = dram.tile([C, 4], F32, tag="st_in_a")
            st_out_a = dram.tile([C, 4], F32, tag="st_out_a")
            st_in_b = dram.tile([C, 4], F32, tag="st_in_b")
            st_out_b = dram.tile([C, 4], F32, tag="st_out_b")

            # ---- attention: per (chunk, key-tile): 4 concurrent row-tiled
            # score MMs; exp split scalar/vector; 4 concurrent col-tiled PV
            # MMs lagged one tile so the PE never waits on exp. The loop is
            # FLAT over all 64 (chunk, tile) pairs so the next chunk's score
            # MMs interleave with the previous chunk's trailing PV flushes —
            # no pipeline drain at chunk boundaries.
            prev = None
            prev_rbc = None
            pends = []
            pv = None

            def flush():
                """PV block for the OLDEST pending tile (its own chunk's
                accumulator rides along in the entry). Two-tile lag: by the
                time these hit the PE queue head, both exp paths have long
                finished, so the in-order queue never stalls."""
                if not pends:
                    return
                p01, p23, t, pvx = pends.pop(0)
                for h in range(H):
                    src = p01 if h < 2 else p23
                    nc.tensor.matmul(
                        pvx[32 * h:32 * h + 32, :],
                        lhsT=vt_sb[:, 128 * t + 32 * h:128 * t + 32 * h + 32],
                        rhs=src[:, 512 * (h % 2):512 * (h % 2) + 512],
                        start=(t == 0), stop=(t == MT - 1),
                        skip_group_check=True,
                        tile_position=(0, 32 * h))

            for T in range(NCH * MT):
                c, t = divmod(T, MT)
                qs = slice(NC * c, NC * (c + 1))
                if t == 0:
                    if pv is not None:
                        prev = (c - 1, pv)
                    pv = ps_pv.tile([C2, NC], F32, tag="pv")
                if True:
                    # t==2, not earlier: the previous chunk's tile-15 PV
                    # flush is emitted during t==1, and the Ln must follow it
                    # in program order (Tile would otherwise order that PV
                    # write after this read)
                    if c > 0 and t == 2 and prev is not None:
                        prev_rbc = epilogue_pre(*prev)
                    if c > 0 and t == 12 and prev is not None:
                        epilogue_post(*prev, prev_rbc)
                        prev = None
                        if c == 2:
                            # chunks 0,1 stats (rows 0:64) are final: reduce
                            # them now, hidden under chunks 2,3 compute
                            nc.sync.dma_start(st_in_a[:], stats[0:C, 0:4])
                            nc.gpsimd.collective_compute(
                                "AllReduce", Alu.add,
                                replica_groups=[list(range(N_CORES))],
                                ins=[st_in_a.opt()], outs=[st_out_a.opt()])
                            nc.sync.dma_start(rb_a[0:C, :], st_out_a[:])
                            nc.gpsimd.tensor_add(fa[0:C, :], rb_a[0:C, 0:2],
                                                 rb_a[0:C, 2:4])
                        if c == 3:
                            # chunk 2's half of the bottom stats can ship
                            # ahead of the tail reduce
                            nc.sync.dma_start(st_in_b[:, 0:2],
                                              stats[C:C2, 0:2])
                    # pool rotation note: with 3 slots and 2 tiles/t, the
                    # sp01-first order gives DVE-freed slots two tiles of
                    # slack and scalar-freed slots one — measured fastest.
                    sp01 = ps_s.tile([C2, 1024], F32, tag="s")
                    sp23 = ps_s.tile([C2, 1024], F32, tag="s")
                    for h in range(H):
                        sp = sp01 if h < 2 else sp23
                        nc.tensor.matmul(
                            sp[:, 512 * (h % 2):512 * (h % 2) + 512],
                            lhsT=k_sb[32 * h:32 * h + 16, 128 * t:128 * (t + 1)],
                            rhs=q_sb[32 * h:32 * h + 16, qs],
                            tile_position=(32 * h, 0))
                    # exp now (other engines), PV one tile behind
                    p01 = pp.tile([C2, 1024], F16, tag="p")
                    nc.scalar.activation(p01[:], sp01[:], Act.Exp,
                                         scale=1.0 / A_EXP)
                    p23 = pp.tile([C2, 1024], F16, tag="p")
                    nc.vector.tensor_scalar(p23[:].bitcast(I16), sp23[:],
                                            1.0, B_EXP,
                                            op0=Alu.mult, op1=Alu.add)
                    if c == NCH - 1 and t >= 10:
                        # keep gpsimd awake near the tail so the collective
                        # trigger doesn't pay its multi-us idle-exit latency
                        nc.gpsimd.tensor_copy(gwk[:], p23[0:1, 0:2])
                    if len(pends) == 2:
                        flush()
                    pends.append((p01, p23, t, pv))
            flush()
            flush()
            prev = (NCH - 1, pv)
            prev_rbc = epilogue_pre(*prev, last=True)
            epilogue_post(*prev, prev_rbc, last=True)

            # ---- reduce the bottom chunk-pair stats (rows 64:128); chunk 2's
            # half shipped during chunk 3, only chunk 3's 256B go here
            nc.sync.dma_start(st_in_b[:, 2:4], stats[C:C2, 2:4])
            nc.gpsimd.collective_compute(
                "AllReduce", Alu.add,
                replica_groups=[list(range(N_CORES))],
                ins=[st_in_b.opt()], outs=[st_out_b.opt()])
            nc.sync.dma_start(rb_b[C:C2, :], st_out_b[:])
            nc.vector.tensor_add(fa[C:C2, :], rb_b[C:C2, 0:2],
                                 rb_b[C:C2, 2:4])
            # fold the channel halves + replicate to 128 partitions in one
            # matmul against the pair-identity matrix
            fps = ps_pv.tile([C2, NC], F32, tag="pv")
            nc.tensor.matmul(fps[:, 0:2], lhsT=fold_sb, rhs=fa[:])

            # ---- finalize: mean/var -> scale/shift. The variance pieces and
            # -gamma*mean run on the (idle) DVE in parallel with the scalar
            # chain, cutting the serial depth to ms->lnv->istd->{sh,sc}.
            ms = sb.tile([C2, 2], F32, tag="ms")
            nc.scalar.activation(ms[:], fps[:, 0:2], Act.Copy, scale=1.0 / CNT)
            msq = sb.tile([C2, 1], F32, tag="msq")
            nc.vector.tensor_mul(msq[:], ms[:, 0:1], ms[:, 0:1])
            var = sb.tile([C2, 1], F32, tag="var")
            nc.vector.scalar_tensor_tensor(var[:], msq[:], -1.0, ms[:, 1:2],
                                           op0=Alu.mult, op1=Alu.add)
            ngm = sb.tile([C2, 1], F32, tag="ngm")
            nc.vector.scalar_tensor_tensor(ngm[:], g_sb[:, 0:1], -1.0,
                                           ms[:, 0:1],
                                           op0=Alu.mult, op1=Alu.mult)
            lnv = sb.tile([C2, 1], F32, tag="lnv")
            nc.scalar.activation(lnv[:], var[:], Act.Ln, bias=eps_t[:, 0:1])
            istd = sb.tile([C2, 1], F32, tag="istd")
            nc.scalar.activation(istd[:], lnv[:], Act.Exp, scale=-0.5)
            sh = sb.tile([C2, 1], F32, tag="sh")
            nc.scalar.activation(sh[:], ngm[:], Act.Identity,
                                 scale=istd[:, 0:1], bias=b_sb[:, 0:1])
            sc = sb.tile([C2, 1], F32, tag="sc")
            nc.scalar.activation(sc[:], g_sb[:, 0:1], Act.Copy,
                                 scale=istd[:, 0:1])

            # ---- BN scale/shift + LeakyReLU as a split Prelu so the first
            # output DMA overlaps the second half's compute
            nc.scalar.activation(yl_sb[:, 0:NC], y_sb[:, 0:NC], Act.Prelu,
                                 scale=sc[:, 0:1], bias=sh[:, 0:1], alpha=LEAK)
            nc.sync.dma_start(out_p[:, 0:512], yl_sb[0:C, 0:NC])
            nc.gpsimd.dma_start(out_p[:, 1024:1536], yl_sb[C:C2, 0:NC])
            nc.scalar.activation(yl_sb[:, NC:2 * NC], y_sb[:, NC:2 * NC],
                                 Act.Prelu,
                                 scale=sc[:, 0:1], bias=sh[:, 0:1], alpha=LEAK)
            nc.sync.dma_start(out_p[:, 512:1024], yl_sb[0:C, NC:2 * NC])
            nc.scalar.dma_start(out_p[:, 1536:N], yl_sb[C:C2, NC:2 * NC])

    nc.compile()

    # Post-compile surgery: one activation table set covers every function
    # used here (Exp, Ln, Copy, Prelu); point the first load at it and drop
    # the rest so the table-load inserter doesn't ping-pong.
    from concourse.hw_specs import get_activation_tables
    tabs = list(get_activation_tables(nc.m.arch).keys())
    nle = tabs.index("natural_log_exp_and_others")
    loads = [(b, i) for b in nc.main_func.blocks for i in b.instructions
             if isinstance(i, mybir.InstLoadActFuncSet)]
    if loads:
        loads[0][1].act_func_set_id = nle
        for b, i in loads[1:]:
            b.instructions.remove(i)
    return nc


_NC_CACHE = None


def _get_nc():
    global _NC_CACHE
    if _NC_CACHE is None:
        _NC_CACHE = build()
    return _NC_CACHE


def _prep_inputs(x_local, x_branch, w_qkv, w_proj, gamma, beta):
    bf16 = ml_dtypes.bfloat16
    x_local = np.asarray(x_local, np.float32)
    x_branch = np.asarray(x_branch, np.float32)
    w_qkv = np.asarray(w_qkv, np.float32)
    w_proj = np.asarray(w_proj, np.float32)
    gamma = np.asarray(gamma, np.float32)
    beta = np.asarray(beta, np.float32)

    X = np.concatenate([x_local, x_branch], axis=1).astype(bf16)  # [B, 128, N]
    WT = w_qkv.T.copy()  # [128, 192]
    wq = np.zeros((C2, 128), np.float32)
    wk = np.zeros((C2, 128), np.float32)
    for h in range(H):
        wq[:, 32 * h:32 * h + D] = WT[:, D * h:D * (h + 1)]
        # fold the softmax scale and the Schraudolph slope into wk
        wk[:, 32 * h:32 * h + D] = WT[:, C + D * h:C + D * (h + 1)] * (A_EXP * SCALE)
    wv = WT[:, 2 * C:3 * C]
    wp = np.zeros((C2, C), np.float32)
    for h in range(H):
        wp[32 * h:32 * h + D, :] = w_proj[:, D * h:D * (h + 1)].T
    fold = np.zeros((C2, 128), np.float32)
    for r in range(C2):
        fold[r, r % 64] = 1.0
        fold[r, r % 64 + 64] = 1.0

    # denom-broadcast selector: out[m, q] = drc[32*(m//32)+16, q]
    bc = np.zeros((C2, 128), np.float32)
    for m in range(128):
        bc[32 * (m // 32) + 16, m] = 1.0

    # packed param blob: bf16 matrices + f32 regions viewed as bf16 pairs
    wpk = np.zeros((C2, PK_COLS), bf16)
    wpk[:, PK_WQ:PK_WQ + 128] = wq.astype(bf16)
    wpk[:, PK_WK:PK_WK + 128] = wk.astype(bf16)
    wpk[:, PK_WV:PK_WV + C] = np.ascontiguousarray(wv).astype(bf16)
    wpk[:, PK_WP:PK_WP + C] = wp.astype(bf16)
    wpk[:, PK_BC:PK_BC + 128] = bc.astype(bf16)

    def pack_f32(col, arr):
        raw = np.ascontiguousarray(arr, np.float32)
        wpk[:, col:col + 2 * raw.shape[1]] = raw.view(np.uint16).view(bf16)

    pack_f32(PK_FOLD, fold)
    g2 = np.concatenate([gamma, gamma]).reshape(C2, 1)
    b2 = np.concatenate([beta, beta]).reshape(C2, 1)
    pack_f32(PK_G, g2)
    pack_f32(PK_B, b2)
    pack_f32(PK_EPS, np.full((C2, 1), BN_EPS, np.float32))

    return [dict(x=np.ascontiguousarray(X[b]), wpk=wpk) for b in range(B)]


def kernel(x_local, x_branch, w_qkv, w_proj, gamma, beta, _trace=False, _tmpdir=None):
    nc = _get_nc()
    in_maps = _prep_inputs(x_local, x_branch, w_qkv, w_proj, gamma, beta)
    res = run_bass_kernel_spmd(nc, in_maps, core_ids=list(range(N_CORES)),
                               trace=_trace, tmpdir=_tmpdir)
    out = np.stack([np.asarray(res.results[i]["out"]) for i in range(N_CORES)])
    if _trace:
        kernel._last_results = res
    return out.astype(np.float32)


# revision 48
# speedup vs baseline: 1.0061x; 1.0061x over previous
"""Fused attention block (QKV conv -> 4-head attention -> proj -> BatchNorm -> LeakyReLU)
distributed over 8 trn2 NeuronCores, data-parallel over the batch dim.

Self-contained: hardcodes shapes B=8, C=64, N=2048, H=4.

Per-core layout tricks (v10 — HAM warm-up, cross-chunk pipelining, fast
tail, AllReduce stats):
  - the attention loop is FLAT over all 64 (chunk, key-tile) pairs: the next
    chunk's score matmuls interleave with the previous chunk's trailing PV
    flushes, so the PE pipeline never drains at chunk boundaries (~4us);
  - a burst of dummy back-to-back matmuls runs during the input-DMA window
    so the PE's HAM clock gate flips to 8/8 (2.4 GHz) before real work;
  - scores computed transposed (S^T = K^T Q, keys on partitions); the 4 heads'
    score matmuls go to 4 distinct PE row-groups (tile_position=(32h,0));
  - PV matmuls go to 4 distinct col-groups (tile_position=(0,32h)),
    accumulating over key tiles in PSUM;
  - exp is split across engines: heads 0,1 take true exp on the scalar
    engine; heads 2,3 take a Schraudolph fast-exp on the vector engine
    (f32->int16 cast of a*s+b via tensor_scalar, bitcast to fp16), with
    the slope a folded into wk host-side; PV runs two tiles behind the
    scores so the in-order PE queue never waits on either exp path;
  - softmax denominators come free from a ones-column in the PV stationary;
    reciprocal via exp(-ln(x)) on the scalar engine; mid-kernel chunks
    broadcast the reciprocal rows via a DRAM round trip (hidden under
    compute), the LAST chunk broadcasts the raw denominators with one
    matmul against a row-selector matrix and takes the reciprocal of the
    result, keeping the critical tail ~6us shorter;
  - all small params ship in ONE packed DMA (bf16 blob, f32 views bitcast);
  - BatchNorm stats cross-core reduced via two [C,4] AllReduces (about half
    the latency of AllGather+local add): the top chunk-pair hidden
    mid-kernel, the bottom pair at the tail with chunk 2's half shipped
    early; a dummy AllReduce at prologue warms the CC stream and tiny
    gpsimd reads keep the trigger path awake;
  - BN finalize: variance pieces and -gamma*mean on the (idle) DVE in
    parallel with the scalar chain; BN scale/shift + LeakyReLU as a split
    Prelu so the first output DMA overlaps the second half's compute.
"""
import numpy as np
import ml_dtypes

import concourse.bass as bass
import concourse.mybir as mybir
from concourse import bacc, tile
from concourse.bass_utils import run_bass_kernel_spmd

B, C, N, H, D = 8, 64, 2048, 4, 16
C2 = 2 * C           # 128 input channels after concat
NC = 512             # query-dim chunk = one fp32 PSUM bank
NCH = N // NC        # 4 chunks
MT = N // 128        # 16 key tiles of 128
F32 = mybir.dt.float32
BF16 = mybir.dt.bfloat16
F16 = mybir.dt.float16
I16 = mybir.dt.int16
SCALE = float(D) ** -0.5
BN_EPS = 1e-5
LEAK = 0.2
N_CORES = 8
CNT = float(B * N)   # batchnorm population count
A_EXP = 1024.0 * 1.4426950408889634   # 2^10 * log2(e)
B_EXP = 15360.0                       # 15 << 10: fp16 exponent bias
N_WARM = 7                            # dummy MMs to flip the HAM clock gate

Alu = mybir.AluOpType
Act = mybir.ActivationFunctionType

# packed-params column layout (bf16 columns)
PK_WQ = 0            # [128,128] bf16
PK_WK = 128          # [128,128] bf16
PK_WV = 256          # [128,64]  bf16
PK_WP = 320          # [128,64]  bf16
PK_FOLD = 384        # [128,128] f32  -> 256 bf16 cols
PK_G = 640           # [128,1] f32 -> 2 cols
PK_B = 642           # [128,1] f32 -> 2 cols
PK_EPS = 644         # [128,1] f32 -> 2 cols
PK_BC = 646          # [128,128] bf16 denom-broadcast selector
PK_COLS = 774


def build():
    nc = bacc.Bacc("TRN2", target_bir_lowering=False, debug=False,
                   num_devices=N_CORES)
    x_p = nc.declare_dram_parameter("x", [C2, N], BF16, isOutput=False)
    wpk_p = nc.declare_dram_parameter("wpk", [C2, PK_COLS], BF16, isOutput=False)
    out_p = nc.declare_dram_parameter("out", [C, N], F32, isOutput=True)

    with tile.TileContext(nc) as tc:
        with (
            tc.tile_pool(name="sb", bufs=1) as sb,
            tc.tile_pool(name="ps_s", bufs=3, space="PSUM") as ps_s,
            tc.tile_pool(name="ps_pv", bufs=2, space="PSUM") as ps_pv,
            tc.tile_pool(name="pp", bufs=8) as pp,
            tc.tile_pool(name="ep", bufs=2) as ep,
            tc.tile_pool(name="dram", bufs=2, space="DRAM") as dram,
        ):
            # ---- persistent SBUF tiles
            x_sb = sb.tile([C2, N], BF16, tag="x")
            wpk_sb = sb.tile([C2, PK_COLS], BF16, tag="wpk")
            wq_sb = wpk_sb[:, PK_WQ:PK_WQ + 128]
            wk_sb = wpk_sb[:, PK_WK:PK_WK + 128]
            wv_sb = wpk_sb[:, PK_WV:PK_WV + C]
            wp_sb = wpk_sb[:, PK_WP:PK_WP + C]
            fold_sb = wpk_sb[:, PK_FOLD:PK_FOLD + 256].bitcast(F32)
            g_sb = wpk_sb[:, PK_G:PK_G + 2].bitcast(F32)
            b_sb = wpk_sb[:, PK_B:PK_B + 2].bitcast(F32)
            eps_t = wpk_sb[:, PK_EPS:PK_EPS + 2].bitcast(F32)
            bc_sb = wpk_sb[:, PK_BC:PK_BC + 128]
            q_sb = sb.tile([C2, N], BF16, tag="q")    # head h rows 32h..32h+16
            k_sb = sb.tile([C2, N], BF16, tag="k")
            # per key-tile, per head: 32 cols = [16 V^T | 1 ones | 15 zeros]
            vt_sb = sb.tile([C2, MT * 128], F16, tag="vt")
            y_sb = sb.tile([C2, 2 * NC], F32, tag="y")  # proj out, fold layout
            yl_sb = sb.tile([C2, 2 * NC], F32, tag="yl")
            stats = sb.tile([C2, 4], F32, tag="stats")
            gwk = sb.tile([1, 2], F16, tag="gwk")   # gpsimd keep-awake scratch
            dmy_sb = sb.tile([8, 2], F32, tag="dmy")
            scr = sb.tile([C2, NC], BF16, tag="scr")  # HAM warm-up operand

            # ---- HAM warm-up: back-to-back dummy matmuls during the input
            # DMA window. ~4us of sustained PE busy flips the clock gate from
            # 4/8 (1.2 GHz) to 8/8 (2.4 GHz); the small inter-MM gaps of the
            # attention body then keep it warm. Nothing consumes the output.
            nc.gpsimd.memset(scr[:], 0.0)
            for w in range(N_WARM):
                wp_ps = ps_pv.tile([C2, NC], F32, tag="pv")
                nc.tensor.matmul(wp_ps[:], lhsT=scr[:, 0:128], rhs=scr[:])

            # ---- prologue loads: x in 512-col pieces so the first QKV
            # matmul starts after 128KB; all small params in ONE packed DMA
            nc.sync.dma_start(x_sb[:, 0:512], x_p[:, 0:512])
            nc.scalar.dma_start(wpk_sb[:], wpk_p[:])
            nc.sync.dma_start(x_sb[:, 512:1024], x_p[:, 512:1024])
            nc.scalar.dma_start(x_sb[:, 1024:1536], x_p[:, 1024:1536])
            nc.sync.dma_start(x_sb[:, 1536:N], x_p[:, 1536:N])

            # V^T zero fill + ones columns on gpsimd (before it blocks on the
            # warm-up collective)
            nc.gpsimd.memset(vt_sb[:], 0.0)
            ones_ap = vt_sb[:].rearrange(
                "q (p h e) -> q p h e", p=MT, h=H, e=32)[:, :, :, 16:17]
            nc.gpsimd.memset(ones_ap, 1.0)

            # ---- QKV projections. q/k evacuated with head h at rows
            # 32h..32h+16 (stationary has zeros elsewhere).
            for c4 in range(4):
                cs = slice(512 * c4, 512 * (c4 + 1))
                qp = ps_pv.tile([C2, NC], F32, tag="pv")
                nc.tensor.matmul(qp[:], lhsT=wq_sb, rhs=x_sb[:, cs])
                nc.scalar.activation(q_sb[:, cs], qp[:], Act.Copy)
                kp = ps_pv.tile([C2, NC], F32, tag="pv")
                nc.tensor.matmul(kp[:], lhsT=wk_sb, rhs=x_sb[:, cs])
                nc.vector.tensor_copy(k_sb[:, cs], kp[:])

            # warm-up AllReduce: wakes the CC stream early so the real one at
            # the tail skips the cold-start latency.
            dm_in = dram.tile([8, 2], F32, tag="dm_in")
            dm_out = dram.tile([8, 2], F32, tag="dm_out")
            nc.gpsimd.memset(dmy_sb[:], 1.0)
            nc.gpsimd.dma_start(dm_in[:], dmy_sb[:])
            nc.gpsimd.collective_compute(
                "AllReduce", Alu.add,
                replica_groups=[list(range(N_CORES))],
                ins=[dm_in.opt()], outs=[dm_out.opt()])

            # all 16 V^T key tiles in ONE psum allocation
            vp_all = ps_s.tile([C2, MT * C], F32, tag="s")
            for p in range(MT):
                nc.tensor.matmul(vp_all[:, C * p:C * (p + 1)],
                                 lhsT=x_sb[:, 128 * p:128 * (p + 1)],
                                 rhs=wv_sb)
            vt_dst = vt_sb[:].rearrange(
                "q (p h e) -> q p h e", p=MT, h=H, e=32)[:, :, :, 0:16]
            vt_src = vp_all[:].rearrange(
                "q (p h d) -> q p h d", p=MT, h=H, d=D)
            nc.vector.tensor_copy(vt_dst, vt_src)

            def epilogue_pre(c, pv, last=False):
                """Denominator chain for chunk c: ln/exp + partition-broadcast.
                Mid-kernel chunks bounce the reciprocal rows through DRAM (the
                round-trip hides behind compute). The LAST chunk sits on the
                critical tail, so it broadcasts via one matmul against a
                row-selector matrix instead (saves the ~8us DMA round trip)."""
                if last:
                    # evacuate pv, broadcast the (positive) denominator rows
                    # to every partition via the selector matmul, then take
                    # the reciprocal of the broadcast result — NaN-free, and
                    # the mul's operands both end up in SBUF
                    pvs = ep.tile([C2, NC], BF16, tag="pvs")
                    nc.scalar.activation(pvs[:], pv[:], Act.Copy)
                    nc.gpsimd.tensor_copy(gwk[:], pvs[0:1, 0:2])
                    rbc_d = ps_s.tile([C2, 2 * NC], F32, tag="s")
                    nc.tensor.matmul(rbc_d[:, 0:NC], lhsT=bc_sb, rhs=pvs[:])
                    dln2 = ep.tile([C2, NC], F32, tag="dln")
                    nc.scalar.activation(dln2[:], rbc_d[:, 0:NC], Act.Ln)
                    rbc = ep.tile([C2, NC], F32, tag="rbc")
                    nc.scalar.activation(rbc[:], dln2[:], Act.Exp, scale=-1.0)
                    nc.gpsimd.tensor_copy(gwk[:], rbc[0:1, 0:2])
                    return (pvs, rbc)
                dln = ep.tile([C2, NC], F32, tag="dln")
                nc.scalar.activation(dln[:], pv[:], Act.Ln)
                drc = ep.tile([C2, NC], F32, tag="drc")
                nc.scalar.activation(drc[:], dln[:], Act.Exp, scale=-1.0)
                rbc = ep.tile([C2, NC], F32, tag="rbc")
                rec_d = dram.tile([H, NC], F32, tag="rec_d")
                for h in range(H):
                    nc.sync.dma_start(rec_d[h:h + 1, :],
                                      drc[32 * h + 16:32 * h + 17, :])
                for h in range(H):
                    nc.sync.dma_start(
                        rbc[32 * h:32 * h + 32, :],
                        rec_d[h:h + 1, :].partition_broadcast(32))
                return rbc

            def epilogue_post(c, pv, rbc, last=False):
                """Normalize chunk-c attention output, project, evac + stats.
                The proj-output PSUM tile comes from the pv ring (the slot the
                normalize just freed) so the scores ring keeps all 3 slots."""
                on = ep.tile([C2, NC], BF16, tag="on")
                if last:
                    pvs, rbc_sb = rbc
                    nc.vector.tensor_mul(on[:], pvs[:], rbc_sb[:])
                    nc.gpsimd.tensor_copy(gwk[:], on[0:1, 0:2])
                else:
                    nc.vector.tensor_mul(on[:], pv[:], rbc[:])
                yp = ps_s.tile([C2, 2 * NC], F32, tag="s")
                r = slice(64 * (c // 2), 64 * (c // 2) + 64)
                nc.tensor.matmul(yp[r, 0:NC], lhsT=wp_sb, rhs=on[:],
                                 tile_position=(0, 64 * (c // 2)))
                ycols = slice(512 * (c % 2), 512 * (c % 2) + 512)
                s0 = 2 * (c % 2)
                nc.vector.tensor_scalar(y_sb[r, ycols], yp[r, 0:NC], 1.0, 0.0,
                                        op0=Alu.mult, op1=Alu.add,
                                        accum_out=stats[r, s0:s0 + 1])
                ysq = ep.tile([C2, NC], F32, tag="dln")
                if last:
                    # critical tail: sumsq on the scalar engine in parallel
                    # with the DVE evac (both read yp from PSUM)
                    nc.scalar.activation(ysq[r, :], yp[r, 0:NC], Act.Square,
                                         accum_out=stats[r, s0 + 1:s0 + 2])
                else:
                    nc.vector.scalar_tensor_tensor(
                        ysq[r, :], y_sb[r, ycols], 0.0, y_sb[r, ycols],
                        op0=Alu.add, op1=Alu.mult,
                        accum_out=stats[r, s0 + 1:s0 + 2])

            # stats-reduce staging (split: top chunk-pair mid-kernel, bottom
            # pair at the tail). AllReduce on [C,4] — roughly half the CC
            # latency of the AllGather-and-add-locally scheme.
            rb_a = sb.tile([C2, 4], F32, tag="rb_a")
            rb_b = sb.tile([C2, 4], F32, tag="rb_b")
            fa = sb.tile([C2, 2], F32, tag="fa")
            st_in_a = dram.tile([C, 4], F32, tag="st_in_a")
            st_out_a = dram.tile([C, 4], F32, tag="st_out_a")
            st_in_b = dram.tile([C, 4], F32, tag="st_in_b")
            st_out_b = dram.tile([C, 4], F32, tag="st_out_b")

            # ---- attention: per (chunk, key-tile): 4 concurrent row-tiled
            # score MMs; exp split scalar/vector; 4 concurrent col-tiled PV
            # MMs lagged one tile so the PE never waits on exp. The loop is
            # FLAT over all 64 (chunk, tile) pairs so the next chunk's score
            # MMs interleave with the previous chunk's trailing PV flushes —
            # no pipeline drain at chunk boundaries.
            prev = None
            prev_rbc = None
            pends = []
            pv = None

            def flush():
                """PV block for the OLDEST pending tile (its own chunk's
                accumulator rides along in the entry). Two-tile lag: by the
                time these hit the PE queue head, both exp paths have long
                finished, so the in-order queue never stalls."""
                if not pends:
                    return
                p01, p23, t, pvx = pends.pop(0)
                for h in range(H):
                    src = p01 if h < 2 else p23
                    nc.tensor.matmul(
                        pvx[32 * h:32 * h + 32, :],
                        lhsT=vt_sb[:, 128 * t + 32 * h:128 * t + 32 * h + 32],
                        rhs=src[:, 512 * (h % 2):512 * (h % 2) + 512],
                        start=(t == 0), stop=(t == MT - 1),
                        skip_group_check=True,
                        tile_position=(0, 32 * h))

            for T in range(NCH * MT):
                c, t = divmod(T, MT)
                qs = slice(NC * c, NC * (c + 1))
                if t == 0:
                    if pv is not None:
                        prev = (c - 1, pv)
                    pv = ps_pv.tile([C2, NC], F32, tag="pv")
                if True:
                    # t==2, not earlier: the previous chunk's tile-15 PV
                    # flush is emitted during t==1, and the Ln must follow it
                    # in program order (Tile would otherwise order that PV
                    # write after this read)
                    if c > 0 and t == 2 and prev is not None:
                        prev_rbc = epilogue_pre(*prev)
                    if c > 0 and t == 12 and prev is not None:
                        epilogue_post(*prev, prev_rbc)
                        prev = None
                        if c == 2:
                            # chunks 0,1 stats (rows 0:64) are final: reduce
                            # them now, hidden under chunks 2,3 compute
                            nc.sync.dma_start(st_in_a[:], stats[0:C, 0:4])
                            nc.gpsimd.collective_compute(
                                "AllReduce", Alu.add,
                                replica_groups=[list(range(N_CORES))],
                                ins=[st_in_a.opt()], outs=[st_out_a.opt()])
                            nc.sync.dma_start(rb_a[0:C, :], st_out_a[:])
                            nc.gpsimd.tensor_add(fa[0:C, :], rb_a[0:C, 0:2],
                                                 rb_a[0:C, 2:4])
                        if c == 3:
                            # chunk 2's half of the bottom stats can ship
                            # ahead of the tail reduce
                            nc.sync.dma_start(st_in_b[:, 0:2],
                                              stats[C:C2, 0:2])
                    # pool rotation note: with 3 slots and 2 tiles/t, the
                    # sp01-first order gives DVE-freed slots two tiles of
                    # slack and scalar-freed slots one — measured fastest.
                    sp01 = ps_s.tile([C2, 1024], F32, tag="s")
                    sp23 = ps_s.tile([C2, 1024], F32, tag="s")
                    for h in range(H):
                        sp = sp01 if h < 2 else sp23
                        nc.tensor.matmul(
                            sp[:, 512 * (h % 2):512 * (h % 2) + 512],
                            lhsT=k_sb[32 * h:32 * h + 16, 128 * t:128 * (t + 1)],
                            rhs=q_sb[32 * h:32 * h + 16, qs],
                            tile_position=(32 * h, 0))
                    # exp now (other engines), PV one tile behind
                    p01 = pp.tile([C2, 1024], F16, tag="p")
                    nc.scalar.activation(p01[:], sp01[:], Act.Exp,
                                         scale=1.0 / A_EXP)
                    p23 = pp.tile([C2, 1024], F16, tag="p")
                    nc.vector.tensor_scalar(p23[:].bitcast(I16), sp23[:],
                                            1.0, B_EXP,
                                            op0=Alu.mult, op1=Alu.add)
                    if c == NCH - 1 and t >= 10:
                        # keep gpsimd awake near the tail so the collective
                        # trigger doesn't pay its multi-us idle-exit latency
                        nc.gpsimd.tensor_copy(gwk[:], p23[0:1, 0:2])
                    if len(pends) == 2:
                        flush()
                    pends.append((p01, p23, t, pv))
            flush()
            flush()
            prev = (NCH - 1, pv)
            prev_rbc = epilogue_pre(*prev, last=True)
            epilogue_post(*prev, prev_rbc, last=True)

            # ---- reduce the bottom chunk-pair stats (rows 64:128); chunk 2's
            # half shipped during chunk 3, only chunk 3's 256B go here
            nc.sync.dma_start(st_in_b[:, 2:4], stats[C:C2, 2:4])
            nc.gpsimd.collective_compute(
                "AllReduce", Alu.add,
                replica_groups=[list(range(N_CORES))],
                ins=[st_in_b.opt()], outs=[st_out_b.opt()])
            nc.sync.dma_start(rb_b[C:C2, :], st_out_b[:])
            nc.vector.tensor_add(fa[C:C2, :], rb_b[C:C2, 0:2],
                                 rb_b[C:C2, 2:4])
            # fold the channel halves + replicate to 128 partitions in one
            # matmul against the pair-identity matrix
            fps = ps_pv.tile([C2, NC], F32, tag="pv")
            nc.tensor.matmul(fps[:, 0:2], lhsT=fold_sb, rhs=fa[:])

            # ---- finalize: mean/var -> scale/shift. The variance pieces and
            # -gamma*mean run on the (idle) DVE in parallel with the scalar
            # chain, cutting the serial depth to ms->lnv->istd->{sh,sc}.
            ms = sb.tile([C2, 2], F32, tag="ms")
            nc.scalar.activation(ms[:], fps[:, 0:2], Act.Copy, scale=1.0 / CNT)
            msq = sb.tile([C2, 1], F32, tag="msq")
            nc.vector.tensor_mul(msq[:], ms[:, 0:1], ms[:, 0:1])
            var = sb.tile([C2, 1], F32, tag="var")
            nc.vector.scalar_tensor_tensor(var[:], msq[:], -1.0, ms[:, 1:2],
                                           op0=Alu.mult, op1=Alu.add)
            ngm = sb.tile([C2, 1], F32, tag="ngm")
            nc.vector.scalar_tensor_tensor(ngm[:], g_sb[:, 0:1], -1.0,
                                           ms[:, 0:1],
                                           op0=Alu.mult, op1=Alu.mult)
            lnv = sb.tile([C2, 1], F32, tag="lnv")
            nc.scalar.activation(lnv[:], var[:], Act.Ln, bias=eps_t[:, 0:1])
            istd = sb.tile([C2, 1], F32, tag="istd")
            nc.scalar.activation(istd[:], lnv[:], Act.Exp, scale=-0.5)
            sh = sb.tile([C2, 1], F32, tag="sh")
            nc.scalar.activation(sh[:], ngm[:], Act.Identity,
                                 scale=istd[:, 0:1], bias=b_sb[:, 0:1])
            sc = sb.tile([C2, 1], F32, tag="sc")
            nc.scalar.activation(sc[:], g_sb[:, 0:1], Act.Copy,
                                 scale=istd[:, 0:1])

            # ---- BN scale/shift + LeakyReLU as a split Prelu so the first
            # output DMA overlaps the second half's compute
            nc.scalar.activation(yl_sb[:, 0:NC], y_sb[:, 0:NC], Act.Prelu,
                                 scale=sc[:, 0:1], bias=sh[:, 0:1], alpha=LEAK)
            nc.sync.dma_start(out_p[:, 0:512], yl_sb[0:C, 0:NC])
            nc.gpsimd.dma_start(out_p[:, 1024:1536], yl_sb[C:C2, 0:NC])
            nc.scalar.activation(yl_sb[:, NC:2 * NC], y_sb[:, NC:2 * NC],
                                 Act.Prelu,
                                 scale=sc[:, 0:1], bias=sh[:, 0:1], alpha=LEAK)
            nc.sync.dma_start(out_p[:, 512:1024], yl_sb[0:C, NC:2 * NC])
            nc.scalar.dma_start(out_p[:, 1536:N], yl_sb[C:C2, NC:2 * NC])

    nc.compile()

    # Post-compile surgery: one activation table set covers every function
    # used here (Exp, Ln, Copy, Prelu); point the first load at it and drop
    # the rest so the table-load inserter doesn't ping-pong.
    from concourse.hw_specs import get_activation_tables
    tabs = list(get_activation_tables(nc.m.arch).keys())
    nle = tabs.index("natural_log_exp_and_others")
    loads = [(b, i) for b in nc.main_func.blocks for i in b.instructions
             if isinstance(i, mybir.InstLoadActFuncSet)]
    if loads:
        loads[0][1].act_func_set_id = nle
        for b, i in loads[1:]:
            b.instructions.remove(i)
    return nc


_NC_CACHE = None


def _get_nc():
    global _NC_CACHE
    if _NC_CACHE is None:
        _NC_CACHE = build()
    return _NC_CACHE


def _prep_inputs(x_local, x_branch, w_qkv, w_proj, gamma, beta):
    bf16 = ml_dtypes.bfloat16
    x_local = np.asarray(x_local, np.float32)
    x_branch = np.asarray(x_branch, np.float32)
    w_qkv = np.asarray(w_qkv, np.float32)
    w_proj = np.asarray(w_proj, np.float32)
    gamma = np.asarray(gamma, np.float32)
    beta = np.asarray(beta, np.float32)

    X = np.concatenate([x_local, x_branch], axis=1).astype(bf16)  # [B, 128, N]
    WT = w_qkv.T.copy()  # [128, 192]
    wq = np.zeros((C2, 128), np.float32)
    wk = np.zeros((C2, 128), np.float32)
    for h in range(H):
        wq[:, 32 * h:32 * h + D] = WT[:, D * h:D * (h + 1)]
        # fold the softmax scale and the Schraudolph slope into wk
        wk[:, 32 * h:32 * h + D] = WT[:, C + D * h:C + D * (h + 1)] * (A_EXP * SCALE)
    wv = WT[:, 2 * C:3 * C]
    wp = np.zeros((C2, C), np.float32)
    for h in range(H):
        wp[32 * h:32 * h + D, :] = w_proj[:, D * h:D * (h + 1)].T
    fold = np.zeros((C2, 128), np.float32)
    for r in range(C2):
        fold[r, r % 64] = 1.0
        fold[r, r % 64 + 64] = 1.0

    # denom-broadcast selector: out[m, q] = drc[32*(m//32)+16, q]
    bc = np.zeros((C2, 128), np.float32)
    for m in range(128):
        bc[32 * (m // 32) + 16, m] = 1.0

    # packed param blob: bf16 matrices + f32 regions viewed as bf16 pairs
    wpk = np.zeros((C2, PK_COLS), bf16)
    wpk[:, PK_WQ:PK_WQ + 128] = wq.astype(bf16)
    wpk[:, PK_WK:PK_WK + 128] = wk.astype(bf16)
    wpk[:, PK_WV:PK_WV + C] = np.ascontiguousarray(wv).astype(bf16)
    wpk[:, PK_WP:PK_WP + C] = wp.astype(bf16)
    wpk[:, PK_BC:PK_BC + 128] = bc.astype(bf16)

    def pack_f32(col, arr):
        raw = np.ascontiguousarray(arr, np.float32)
        wpk[:, col:col + 2 * raw.shape[1]] = raw.view(np.uint16).view(bf16)

    pack_f32(PK_FOLD, fold)
    g2 = np.concatenate([gamma, gamma]).reshape(C2, 1)
    b2 = np.concatenate([beta, beta]).reshape(C2, 1)
    pack_f32(PK_G, g2)
    pack_f32(PK_B, b2)
    pack_f32(PK_EPS, np.full((C2, 1), BN_EPS, np.float32))

    return [dict(x=np.ascontiguousarray(X[b]), wpk=wpk) for b in range(B)]


def kernel(x_local, x_branch, w_qkv, w_proj, gamma, beta, _trace=False, _tmpdir=None):
    nc = _get_nc()
    in_maps = _prep_inputs(x_local, x_branch, w_qkv, w_proj, gamma, beta)
    res = run_bass_kernel_spmd(nc, in_maps, core_ids=list(range(N_CORES)),
                               trace=_trace, tmpdir=_tmpdir)
    out = np.stack([np.asarray(res.results[i]["out"]) for i in range(N_CORES)])
    if _trace:
        kernel._last_results = res
    return out.astype(np.float32)


# revision 50
# speedup vs baseline: 1.0138x; 1.0077x over previous
"""Fused attention block (QKV conv -> 4-head attention -> proj -> BatchNorm -> LeakyReLU)
distributed over 8 trn2 NeuronCores, data-parallel over the batch dim.

Self-contained: hardcodes shapes B=8, C=64, N=2048, H=4.

Per-core layout tricks (v10 — HAM warm-up, cross-chunk pipelining, fast
tail, AllReduce stats):
  - the attention loop is FLAT over all 64 (chunk, key-tile) pairs: the next
    chunk's score matmuls interleave with the previous chunk's trailing PV
    flushes, so the PE pipeline never drains at chunk boundaries (~4us);
  - a burst of dummy back-to-back matmuls runs during the input-DMA window
    so the PE's HAM clock gate flips to 8/8 (2.4 GHz) before real work;
  - scores computed transposed (S^T = K^T Q, keys on partitions); the 4 heads'
    score matmuls go to 4 distinct PE row-groups (tile_position=(32h,0));
  - PV matmuls go to 4 distinct col-groups (tile_position=(0,32h)),
    accumulating over key tiles in PSUM;
  - exp is split across engines: heads 0,1 take true exp on the scalar
    engine; heads 2,3 take a Schraudolph fast-exp on the vector engine
    (f32->int16 cast of a*s+b via tensor_scalar, bitcast to fp16), with
    the slope a folded into wk host-side; PV runs two tiles behind the
    scores so the in-order PE queue never waits on either exp path;
  - softmax denominators come free from a ones-column in the PV stationary;
    reciprocal via exp(-ln(x)) on the scalar engine; mid-kernel chunks
    broadcast the reciprocal rows via a DRAM round trip (hidden under
    compute), the LAST chunk broadcasts the raw denominators with one
    matmul against a row-selector matrix and takes the reciprocal of the
    result, keeping the critical tail ~6us shorter;
  - all small params ship in ONE packed DMA (bf16 blob, f32 views bitcast);
  - BatchNorm stats cross-core reduced via two [C,4] AllReduces (about half
    the latency of AllGather+local add): the top chunk-pair hidden
    mid-kernel, the bottom pair at the tail with chunk 2's half shipped
    early; a dummy AllReduce at prologue warms the CC stream and tiny
    gpsimd reads keep the trigger path awake;
  - BN finalize: variance pieces and -gamma*mean on the (idle) DVE in
    parallel with the scalar chain; BN scale/shift + LeakyReLU as a split
    Prelu so the first output DMA overlaps the second half's compute.
"""
import numpy as np
import ml_dtypes

import concourse.bass as bass
import concourse.mybir as mybir
from concourse import bacc, tile
from concourse.bass_utils import run_bass_kernel_spmd

B, C, N, H, D = 8, 64, 2048, 4, 16
C2 = 2 * C           # 128 input channels after concat
NC = 512             # query-dim chunk = one fp32 PSUM bank
NCH = N // NC        # 4 chunks
MT = N // 128        # 16 key tiles of 128
F32 = mybir.dt.float32
BF16 = mybir.dt.bfloat16
F16 = mybir.dt.float16
I16 = mybir.dt.int16
SCALE = float(D) ** -0.5
BN_EPS = 1e-5
LEAK = 0.2
N_CORES = 8
CNT = float(B * N)   # batchnorm population count
A_EXP = 1024.0 * 1.4426950408889634   # 2^10 * log2(e)
B_EXP = 15360.0                       # 15 << 10: fp16 exponent bias
N_WARM = 7                            # dummy MMs to flip the HAM clock gate

Alu = mybir.AluOpType
Act = mybir.ActivationFunctionType

# packed-params column layout (bf16 columns)
PK_WQ = 0            # [128,128] bf16
PK_WK = 128          # [128,128] bf16
PK_WV = 256          # [128,64]  bf16
PK_WP = 320          # [128,64]  bf16
PK_FOLD = 384        # [128,128] f32  -> 256 bf16 cols
PK_G = 640           # [128,1] f32 -> 2 cols
PK_B = 642           # [128,1] f32 -> 2 cols
PK_EPS = 644         # [128,1] f32 -> 2 cols
PK_BC = 646          # [128,128] bf16 denom-broadcast selector
PK_COLS = 774


def build():
    nc = bacc.Bacc("TRN2", target_bir_lowering=False, debug=False,
                   num_devices=N_CORES)
    x_p = nc.declare_dram_parameter("x", [C2, N], BF16, isOutput=False)
    wpk_p = nc.declare_dram_parameter("wpk", [C2, PK_COLS], BF16, isOutput=False)
    out_p = nc.declare_dram_parameter("out", [C, N], F32, isOutput=True)

    with tile.TileContext(nc) as tc:
        with (
            tc.tile_pool(name="sb", bufs=1) as sb,
            tc.tile_pool(name="ps_s", bufs=3, space="PSUM") as ps_s,
            tc.tile_pool(name="ps_pv", bufs=2, space="PSUM") as ps_pv,
            tc.tile_pool(name="pp", bufs=8) as pp,
            tc.tile_pool(name="ep", bufs=2) as ep,
            tc.tile_pool(name="dram", bufs=2, space="DRAM") as dram,
        ):
            # ---- persistent SBUF tiles
            x_sb = sb.tile([C2, N], BF16, tag="x")
            wpk_sb = sb.tile([C2, PK_COLS], BF16, tag="wpk")
            wq_sb = wpk_sb[:, PK_WQ:PK_WQ + 128]
            wk_sb = wpk_sb[:, PK_WK:PK_WK + 128]
            wv_sb = wpk_sb[:, PK_WV:PK_WV + C]
            wp_sb = wpk_sb[:, PK_WP:PK_WP + C]
            fold_sb = wpk_sb[:, PK_FOLD:PK_FOLD + 256].bitcast(F32)
            g_sb = wpk_sb[:, PK_G:PK_G + 2].bitcast(F32)
            b_sb = wpk_sb[:, PK_B:PK_B + 2].bitcast(F32)
            eps_t = wpk_sb[:, PK_EPS:PK_EPS + 2].bitcast(F32)
            bc_sb = wpk_sb[:, PK_BC:PK_BC + 128]
            q_sb = sb.tile([C2, N], BF16, tag="q")    # head h rows 32h..32h+16
            k_sb = sb.tile([C2, N], BF16, tag="k")
            # per key-tile, per head: 32 cols = [16 V^T | 1 ones | 15 zeros]
            vt_sb = sb.tile([C2, MT * 128], F16, tag="vt")
            y_sb = sb.tile([C2, 2 * NC], F32, tag="y")  # proj out, fold layout
            yl_sb = sb.tile([C2, 2 * NC], F32, tag="yl")
            stats = sb.tile([C2, 4], F32, tag="stats")
            gwk = sb.tile([1, 2], F16, tag="gwk")   # gpsimd keep-awake scratch
            dmy_sb = sb.tile([8, 2], F32, tag="dmy")
            scr = sb.tile([C2, NC], BF16, tag="scr")  # HAM warm-up operand

            # ---- HAM warm-up: back-to-back dummy matmuls during the input
            # DMA window. ~4us of sustained PE busy flips the clock gate from
            # 4/8 (1.2 GHz) to 8/8 (2.4 GHz); the small inter-MM gaps of the
            # attention body then keep it warm. Nothing consumes the output.
            nc.gpsimd.memset(scr[:], 0.0)
            for w in range(N_WARM):
                wp_ps = ps_pv.tile([C2, NC], F32, tag="pv")
                nc.tensor.matmul(wp_ps[:], lhsT=scr[:, 0:128], rhs=scr[:])

            # ---- prologue loads: x in 512-col pieces so the first QKV
            # matmul starts after 128KB; all small params in ONE packed DMA
            nc.sync.dma_start(x_sb[:, 0:512], x_p[:, 0:512])
            nc.scalar.dma_start(wpk_sb[:], wpk_p[:])
            nc.sync.dma_start(x_sb[:, 512:1024], x_p[:, 512:1024])
            nc.scalar.dma_start(x_sb[:, 1024:1536], x_p[:, 1024:1536])
            nc.sync.dma_start(x_sb[:, 1536:N], x_p[:, 1536:N])

            # V^T zero fill + ones columns on gpsimd (before it blocks on the
            # warm-up collective)
            nc.gpsimd.memset(vt_sb[:], 0.0)
            ones_ap = vt_sb[:].rearrange(
                "q (p h e) -> q p h e", p=MT, h=H, e=32)[:, :, :, 16:17]
            nc.gpsimd.memset(ones_ap, 1.0)

            # ---- QKV projections. q/k evacuated with head h at rows
            # 32h..32h+16 (stationary has zeros elsewhere).
            for c4 in range(4):
                cs = slice(512 * c4, 512 * (c4 + 1))
                qp = ps_pv.tile([C2, NC], F32, tag="pv")
                nc.tensor.matmul(qp[:], lhsT=wq_sb, rhs=x_sb[:, cs])
                nc.scalar.activation(q_sb[:, cs], qp[:], Act.Copy)
                kp = ps_pv.tile([C2, NC], F32, tag="pv")
                nc.tensor.matmul(kp[:], lhsT=wk_sb, rhs=x_sb[:, cs])
                nc.vector.tensor_copy(k_sb[:, cs], kp[:])

            # warm-up AllReduce: wakes the CC stream early so the real one at
            # the tail skips the cold-start latency.
            dm_in = dram.tile([8, 2], F32, tag="dm_in")
            dm_out = dram.tile([8, 2], F32, tag="dm_out")
            nc.gpsimd.memset(dmy_sb[:], 1.0)
            nc.gpsimd.dma_start(dm_in[:], dmy_sb[:])
            nc.gpsimd.collective_compute(
                "AllReduce", Alu.add,
                replica_groups=[list(range(N_CORES))],
                ins=[dm_in.opt()], outs=[dm_out.opt()])

            # all 16 V^T key tiles in ONE psum allocation
            vp_all = ps_s.tile([C2, MT * C], F32, tag="s")
            for p in range(MT):
                nc.tensor.matmul(vp_all[:, C * p:C * (p + 1)],
                                 lhsT=x_sb[:, 128 * p:128 * (p + 1)],
                                 rhs=wv_sb)
            vt_dst = vt_sb[:].rearrange(
                "q (p h e) -> q p h e", p=MT, h=H, e=32)[:, :, :, 0:16]
            vt_src = vp_all[:].rearrange(
                "q (p h d) -> q p h d", p=MT, h=H, d=D)
            nc.vector.tensor_copy(vt_dst, vt_src)

            def epilogue_pre(c, pv, last=False):
                """Denominator chain for chunk c: ln/exp + partition-broadcast.
                Mid-kernel chunks bounce the reciprocal rows through DRAM (the
                round-trip hides behind compute). The LAST chunk sits on the
                critical tail, so it broadcasts via one matmul against a
                row-selector matrix instead (saves the ~8us DMA round trip)."""
                if last:
                    # evacuate pv, broadcast the (positive) denominator rows
                    # to every partition via the selector matmul, then take
                    # the reciprocal of the broadcast result — NaN-free, and
                    # the mul's operands both end up in SBUF
                    pvs = ep.tile([C2, NC], BF16, tag="pvs")
                    nc.scalar.activation(pvs[:], pv[:], Act.Copy)
                    nc.gpsimd.tensor_copy(gwk[:], pvs[0:1, 0:2])
                    rbc_d = ps_s.tile([C2, 2 * NC], F32, tag="s")
                    nc.tensor.matmul(rbc_d[:, 0:NC], lhsT=bc_sb, rhs=pvs[:])
                    dln2 = ep.tile([C2, NC], F32, tag="dln")
                    nc.scalar.activation(dln2[:], rbc_d[:, 0:NC], Act.Ln)
                    rbc = ep.tile([C2, NC], F32, tag="rbc")
                    nc.scalar.activation(rbc[:], dln2[:], Act.Exp, scale=-1.0)
                    nc.gpsimd.tensor_copy(gwk[:], rbc[0:1, 0:2])
                    return (pvs, rbc)
                dln = ep.tile([C2, NC], F32, tag="dln")
                nc.scalar.activation(dln[:], pv[:], Act.Ln)
                drc = ep.tile([C2, NC], F32, tag="drc")
                nc.scalar.activation(drc[:], dln[:], Act.Exp, scale=-1.0)
                rbc = ep.tile([C2, NC], F32, tag="rbc")
                rec_d = dram.tile([H, NC], F32, tag="rec_d")
                for h in range(H):
                    nc.sync.dma_start(rec_d[h:h + 1, :],
                                      drc[32 * h + 16:32 * h + 17, :])
                for h in range(H):
                    nc.sync.dma_start(
                        rbc[32 * h:32 * h + 32, :],
                        rec_d[h:h + 1, :].partition_broadcast(32))
                return rbc

            def epilogue_post(c, pv, rbc, last=False):
                """Normalize chunk-c attention output, project, evac + stats.
                The proj-output PSUM tile comes from the pv ring (the slot the
                normalize just freed) so the scores ring keeps all 3 slots."""
                on = ep.tile([C2, NC], BF16, tag="on")
                if last:
                    pvs, rbc_sb = rbc
                    nc.vector.tensor_mul(on[:], pvs[:], rbc_sb[:])
                    nc.gpsimd.tensor_copy(gwk[:], on[0:1, 0:2])
                else:
                    nc.vector.tensor_mul(on[:], pv[:], rbc[:])
                yp = ps_s.tile([C2, 2 * NC], F32, tag="s")
                r = slice(64 * (c // 2), 64 * (c // 2) + 64)
                nc.tensor.matmul(yp[r, 0:NC], lhsT=wp_sb, rhs=on[:],
                                 tile_position=(0, 64 * (c // 2)))
                ycols = slice(512 * (c % 2), 512 * (c % 2) + 512)
                s0 = 2 * (c % 2)
                nc.vector.tensor_scalar(y_sb[r, ycols], yp[r, 0:NC], 1.0, 0.0,
                                        op0=Alu.mult, op1=Alu.add,
                                        accum_out=stats[r, s0:s0 + 1])
                ysq = ep.tile([C2, NC], F32, tag="dln")
                if last:
                    # critical tail: sumsq on the scalar engine in parallel
                    # with the DVE evac (both read yp from PSUM)
                    nc.scalar.activation(ysq[r, :], yp[r, 0:NC], Act.Square,
                                         accum_out=stats[r, s0 + 1:s0 + 2])
                else:
                    nc.vector.scalar_tensor_tensor(
                        ysq[r, :], y_sb[r, ycols], 0.0, y_sb[r, ycols],
                        op0=Alu.add, op1=Alu.mult,
                        accum_out=stats[r, s0 + 1:s0 + 2])

            # stats-reduce staging (split: top chunk-pair mid-kernel, bottom
            # pair at the tail). AllReduce on [C,4] — roughly half the CC
            # latency of the AllGather-and-add-locally scheme.
            rb_a = sb.tile([C2, 4], F32, tag="rb_a")
            rb_b = sb.tile([C2, 4], F32, tag="rb_b")
            fa = sb.tile([C2, 2], F32, tag="fa")
            st_in_a = dram.tile([C, 4], F32, tag="st_in_a")
            st_out_a = dram.tile([C, 4], F32, tag="st_out_a")
            st_in_b = dram.tile([C, 4], F32, tag="st_in_b")
            st_out_b = dram.tile([C, 4], F32, tag="st_out_b")

            # ---- attention: per (chunk, key-tile): 4 concurrent row-tiled
            # score MMs; exp split scalar/vector; 4 concurrent col-tiled PV
            # MMs lagged one tile so the PE never waits on exp. The loop is
            # FLAT over all 64 (chunk, tile) pairs so the next chunk's score
            # MMs interleave with the previous chunk's trailing PV flushes —
            # no pipeline drain at chunk boundaries.
            prev = None
            prev_rbc = None
            pends = []
            pv = None

            def flush():
                """PV block for the OLDEST pending tile (its own chunk's
                accumulator rides along in the entry). Two-tile lag: by the
                time these hit the PE queue head, both exp paths have long
                finished, so the in-order queue never stalls."""
                if not pends:
                    return
                p01, p23, t, pvx = pends.pop(0)
                for h in range(H):
                    src = p01 if h < 2 else p23
                    nc.tensor.matmul(
                        pvx[32 * h:32 * h + 32, :],
                        lhsT=vt_sb[:, 128 * t + 32 * h:128 * t + 32 * h + 32],
                        rhs=src[:, 512 * (h % 2):512 * (h % 2) + 512],
                        start=(t == 0), stop=(t == MT - 1),
                        skip_group_check=True,
                        tile_position=(0, 32 * h))

            for T in range(NCH * MT):
                c, t = divmod(T, MT)
                qs = slice(NC * c, NC * (c + 1))
                if t == 0:
                    if pv is not None:
                        prev = (c - 1, pv)
                    pv = ps_pv.tile([C2, NC], F32, tag="pv")
                if True:
                    # t==2, not earlier: the previous chunk's tile-15 PV
                    # flush is emitted during t==1, and the Ln must follow it
                    # in program order (Tile would otherwise order that PV
                    # write after this read)
                    if c > 0 and t == 2 and prev is not None:
                        prev_rbc = epilogue_pre(*prev)
                    if c > 0 and t == 12 and prev is not None:
                        epilogue_post(*prev, prev_rbc)
                        prev = None
                        if c == 2:
                            # chunks 0,1 stats (rows 0:64) are final: reduce
                            # them now, hidden under chunks 2,3 compute
                            nc.sync.dma_start(st_in_a[:], stats[0:C, 0:4])
                            nc.gpsimd.collective_compute(
                                "AllReduce", Alu.add,
                                replica_groups=[list(range(N_CORES))],
                                ins=[st_in_a.opt()], outs=[st_out_a.opt()])
                            nc.sync.dma_start(rb_a[0:C, :], st_out_a[:])
                            nc.gpsimd.tensor_add(fa[0:C, :], rb_a[0:C, 0:2],
                                                 rb_a[0:C, 2:4])
                        if c == 3:
                            # chunk 2's half of the bottom stats can ship
                            # ahead of the tail reduce
                            nc.sync.dma_start(st_in_b[:, 0:2],
                                              stats[C:C2, 0:2])
                    # pool rotation note: with 3 slots and 2 tiles/t, the
                    # sp01-first order gives DVE-freed slots two tiles of
                    # slack and scalar-freed slots one — measured fastest.
                    sp01 = ps_s.tile([C2, 1024], F32, tag="s")
                    sp23 = ps_s.tile([C2, 1024], F32, tag="s")
                    for h in range(H):
                        sp = sp01 if h < 2 else sp23
                        nc.tensor.matmul(
                            sp[:, 512 * (h % 2):512 * (h % 2) + 512],
                            lhsT=k_sb[32 * h:32 * h + 16, 128 * t:128 * (t + 1)],
                            rhs=q_sb[32 * h:32 * h + 16, qs],
                            tile_position=(32 * h, 0))
                    # exp now (other engines), PV one tile behind
                    p01 = pp.tile([C2, 1024], F16, tag="p")
                    nc.scalar.activation(p01[:], sp01[:], Act.Exp,
                                         scale=1.0 / A_EXP)
                    p23 = pp.tile([C2, 1024], F16, tag="p")
                    nc.vector.tensor_scalar(p23[:].bitcast(I16), sp23[:],
                                            1.0, B_EXP,
                                            op0=Alu.mult, op1=Alu.add)
                    if c == NCH - 1 and t >= 10:
                        # keep gpsimd awake near the tail so the collective
                        # trigger doesn't pay its multi-us idle-exit latency
                        nc.gpsimd.tensor_copy(gwk[:], p23[0:1, 0:2])
                    if len(pends) == 2:
                        flush()
                    pends.append((p01, p23, t, pv))
            flush()
            flush()
            prev = (NCH - 1, pv)
            prev_rbc = epilogue_pre(*prev, last=True)
            epilogue_post(*prev, prev_rbc, last=True)

            # ---- reduce the bottom chunk-pair stats (rows 64:128); chunk 2's
            # half shipped during chunk 3, only chunk 3's 256B go here
            nc.sync.dma_start(st_in_b[:, 2:4], stats[C:C2, 2:4])
            nc.gpsimd.collective_compute(
                "AllReduce", Alu.add,
                replica_groups=[list(range(N_CORES))],
                ins=[st_in_b.opt()], outs=[st_out_b.opt()])
            nc.sync.dma_start(rb_b[C:C2, :], st_out_b[:])
            nc.vector.tensor_add(fa[C:C2, :], rb_b[C:C2, 0:2],
                                 rb_b[C:C2, 2:4])
            # fold the channel halves + replicate to 128 partitions in one
            # matmul against the pair-identity matrix
            fps = ps_pv.tile([C2, NC], F32, tag="pv")
            nc.tensor.matmul(fps[:, 0:2], lhsT=fold_sb, rhs=fa[:])

            # ---- finalize: mean/var -> scale/shift. The variance pieces and
            # -gamma*mean run on the (idle) DVE in parallel with the scalar
            # chain, cutting the serial depth to ms->lnv->istd->{sh,sc}.
            ms = sb.tile([C2, 2], F32, tag="ms")
            nc.scalar.activation(ms[:], fps[:, 0:2], Act.Copy, scale=1.0 / CNT)
            msq = sb.tile([C2, 1], F32, tag="msq")
            nc.vector.tensor_mul(msq[:], ms[:, 0:1], ms[:, 0:1])
            var = sb.tile([C2, 1], F32, tag="var")
            nc.vector.scalar_tensor_tensor(var[:], msq[:], -1.0, ms[:, 1:2],
                                           op0=Alu.mult, op1=Alu.add)
            ngm = sb.tile([C2, 1], F32, tag="ngm")
            nc.vector.scalar_tensor_tensor(ngm[:], g_sb[:, 0:1], -1.0,
                                           ms[:, 0:1],
                                           op0=Alu.mult, op1=Alu.mult)
            lnv = sb.tile([C2, 1], F32, tag="lnv")
            nc.scalar.activation(lnv[:], var[:], Act.Ln, bias=eps_t[:, 0:1])
            istd = sb.tile([C2, 1], F32, tag="istd")
            nc.scalar.activation(istd[:], lnv[:], Act.Exp, scale=-0.5)
            sh = sb.tile([C2, 1], F32, tag="sh")
            nc.scalar.activation(sh[:], ngm[:], Act.Identity,
                                 scale=istd[:, 0:1], bias=b_sb[:, 0:1])
            sc = sb.tile([C2, 1], F32, tag="sc")
            nc.scalar.activation(sc[:], g_sb[:, 0:1], Act.Copy,
                                 scale=istd[:, 0:1])

            # ---- BN scale/shift + LeakyReLU as a split Prelu so the first
            # output DMA overlaps the second half's compute
            nc.scalar.activation(yl_sb[:, 0:NC], y_sb[:, 0:NC], Act.Prelu,
                                 scale=sc[:, 0:1], bias=sh[:, 0:1], alpha=LEAK)
            nc.sync.dma_start(out_p[:, 0:512], yl_sb[0:C, 0:NC])
            nc.gpsimd.dma_start(out_p[:, 1024:1536], yl_sb[C:C2, 0:NC])
            nc.scalar.activation(yl_sb[:, NC:2 * NC], y_sb[:, NC:2 * NC],
                                 Act.Prelu,
                                 scale=sc[:, 0:1], bias=sh[:, 0:1], alpha=LEAK)
            nc.sync.dma_start(out_p[:, 512:1024], yl_sb[0:C, NC:2 * NC])
            nc.scalar.dma_start(out_p[:, 1536:N], yl_sb[C:C2, NC:2 * NC])

    nc.compile()

    # Post-compile surgery: one activation table set covers every function
    # used here (Exp, Ln, Copy, Prelu); point the first load at it and drop
    # the rest so the table-load inserter doesn't ping-pong.
    from concourse.hw_specs import get_activation_tables
    tabs = list(get_activation_tables(nc.m.arch).keys())
    nle = tabs.index("natural_log_exp_and_others")
    loads = [(b, i) for b in nc.main_func.blocks for i in b.instructions
             if isinstance(i, mybir.InstLoadActFuncSet)]
    if loads:
        loads[0][1].act_func_set_id = nle
        for b, i in loads[1:]:
            b.instructions.remove(i)
    return nc


_NC_CACHE = None


def _get_nc():
    global _NC_CACHE
    if _NC_CACHE is None:
        _NC_CACHE = build()
    return _NC_CACHE


def _prep_inputs(x_local, x_branch, w_qkv, w_proj, gamma, beta):
    bf16 = ml_dtypes.bfloat16
    x_local = np.asarray(x_local, np.float32)
    x_branch = np.asarray(x_branch, np.float32)
    w_qkv = np.asarray(w_qkv, np.float32)
    w_proj = np.asarray(w_proj, np.float32)
    gamma = np.asarray(gamma, np.float32)
    beta = np.asarray(beta, np.float32)

    X = np.concatenate([x_local, x_branch], axis=1).astype(bf16)  # [B, 128, N]
    WT = w_qkv.T.copy()  # [128, 192]
    wq = np.zeros((C2, 128), np.float32)
    wk = np.zeros((C2, 128), np.float32)
    for h in range(H):
        wq[:, 32 * h:32 * h + D] = WT[:, D * h:D * (h + 1)]
        # fold the softmax scale and the Schraudolph slope into wk
        wk[:, 32 * h:32 * h + D] = WT[:, C + D * h:C + D * (h + 1)] * (A_EXP * SCALE)
    wv = WT[:, 2 * C:3 * C]
    wp = np.zeros((C2, C), np.float32)
    for h in range(H):
        wp[32 * h:32 * h + D, :] = w_proj[:, D * h:D * (h + 1)].T
    fold = np.zeros((C2, 128), np.float32)
    for r in range(C2):
        fold[r, r % 64] = 1.0
        fold[r, r % 64 + 64] = 1.0

    # denom-broadcast selector: out[m, q] = drc[32*(m//32)+16, q]
    bc = np.zeros((C2, 128), np.float32)
    for m in range(128):
        bc[32 * (m // 32) + 16, m] = 1.0

    # packed param blob: bf16 matrices + f32 regions viewed as bf16 pairs
    wpk = np.zeros((C2, PK_COLS), bf16)
    wpk[:, PK_WQ:PK_WQ + 128] = wq.astype(bf16)
    wpk[:, PK_WK:PK_WK + 128] = wk.astype(bf16)
    wpk[:, PK_WV:PK_WV + C] = np.ascontiguousarray(wv).astype(bf16)
    wpk[:, PK_WP:PK_WP + C] = wp.astype(bf16)
    wpk[:, PK_BC:PK_BC + 128] = bc.astype(bf16)

    def pack_f32(col, arr):
        raw = np.ascontiguousarray(arr, np.float32)
        wpk[:, col:col + 2 * raw.shape[1]] = raw.view(np.uint16).view(bf16)

    pack_f32(PK_FOLD, fold)
    g2 = np.concatenate([gamma, gamma]).reshape(C2, 1)
    b2 = np.concatenate([beta, beta]).reshape(C2, 1)
    pack_f32(PK_G, g2)
    pack_f32(PK_B, b2)
    pack_f32(PK_EPS, np.full((C2, 1), BN_EPS, np.float32))

    return [dict(x=np.ascontiguousarray(X[b]), wpk=wpk) for b in range(B)]


def kernel(x_local, x_branch, w_qkv, w_proj, gamma, beta, _trace=False, _tmpdir=None):
    nc = _get_nc()
    in_maps = _prep_inputs(x_local, x_branch, w_qkv, w_proj, gamma, beta)
    res = run_bass_kernel_spmd(nc, in_maps, core_ids=list(range(N_CORES)),
                               trace=_trace, tmpdir=_tmpdir)
    out = np.stack([np.asarray(res.results[i]["out"]) for i in range(N_CORES)])
    if _trace:
        kernel._last_results = res
    return out.astype(np.float32)


# revision 51
# speedup vs baseline: 1.0978x; 1.0828x over previous
"""Fused attention block (QKV conv -> 4-head attention -> proj -> BatchNorm -> LeakyReLU)
distributed over 8 trn2 NeuronCores, data-parallel over the batch dim.

Self-contained: hardcodes shapes B=8, C=64, N=2048, H=4.

Per-core layout tricks (v10 — HAM warm-up, cross-chunk pipelining, fast
tail, AllReduce stats):
  - the attention loop is FLAT over all 64 (chunk, key-tile) pairs: the next
    chunk's score matmuls interleave with the previous chunk's trailing PV
    flushes, so the PE pipeline never drains at chunk boundaries (~4us);
  - a burst of dummy back-to-back matmuls runs during the input-DMA window
    so the PE's HAM clock gate flips to 8/8 (2.4 GHz) before real work;
  - scores computed transposed (S^T = K^T Q, keys on partitions); the 4 heads'
    score matmuls go to 4 distinct PE row-groups (tile_position=(32h,0));
  - PV matmuls go to 4 distinct col-groups (tile_position=(0,32h)),
    accumulating over key tiles in PSUM;
  - exp is split across engines: heads 0,1 take true exp on the scalar
    engine; heads 2,3 take a Schraudolph fast-exp on the vector engine
    (f32->int16 cast of a*s+b via tensor_scalar, bitcast to fp16), with
    the slope a folded into wk host-side; PV runs two tiles behind the
    scores so the in-order PE queue never waits on either exp path;
  - softmax denominators come free from a ones-column in the PV stationary;
    reciprocal via exp(-ln(x)) on the scalar engine; mid-kernel chunks
    broadcast the reciprocal rows via a DRAM round trip (hidden under
    compute), the LAST chunk broadcasts the raw denominators with one
    matmul against a row-selector matrix and takes the reciprocal of the
    result, keeping the critical tail ~6us shorter;
  - all small params ship in ONE packed DMA (bf16 blob, f32 views bitcast);
  - BatchNorm stats cross-core reduced via two [C,4] AllReduces (about half
    the latency of AllGather+local add): the top chunk-pair hidden
    mid-kernel, the bottom pair at the tail with chunk 2's half shipped
    early; a dummy AllReduce at prologue warms the CC stream and tiny
    gpsimd reads keep the trigger path awake;
  - BN finalize: variance pieces and -gamma*mean on the (idle) DVE in
    parallel with the scalar chain; BN scale/shift + LeakyReLU as a split
    Prelu so the first output DMA overlaps the second half's compute.
"""
import numpy as np
import ml_dtypes

import concourse.bass as bass
import concourse.mybir as mybir
from concourse import bacc, tile
from concourse.bass_utils import run_bass_kernel_spmd

B, C, N, H, D = 8, 64, 2048, 4, 16
C2 = 2 * C           # 128 input channels after concat
NC = 512             # query-dim chunk = one fp32 PSUM bank
NCH = N // NC        # 4 chunks
MT = N // 128        # 16 key tiles of 128
F32 = mybir.dt.float32
BF16 = mybir.dt.bfloat16
F16 = mybir.dt.float16
I16 = mybir.dt.int16
SCALE = float(D) ** -0.5
BN_EPS = 1e-5
LEAK = 0.2
N_CORES = 8
CNT = float(B * N)   # batchnorm population count
A_EXP = 1024.0 * 1.4426950408889634   # 2^10 * log2(e)
B_EXP = 15360.0                       # 15 << 10: fp16 exponent bias
N_WARM = 9                            # dummy MMs to flip the HAM clock gate

Alu = mybir.AluOpType
Act = mybir.ActivationFunctionType

# packed-params column layout (bf16 columns)
PK_WQ = 0            # [128,128] bf16
PK_WK = 128          # [128,128] bf16
PK_WV = 256          # [128,64]  bf16
PK_WP = 320          # [128,64]  bf16
PK_FOLD = 384        # [128,128] f32  -> 256 bf16 cols
PK_G = 640           # [128,1] f32 -> 2 cols
PK_B = 642           # [128,1] f32 -> 2 cols
PK_EPS = 644         # [128,1] f32 -> 2 cols
PK_BC = 646          # [128,128] bf16 denom-broadcast selector
PK_COLS = 774


def build():
    nc = bacc.Bacc("TRN2", target_bir_lowering=False, debug=False,
                   num_devices=N_CORES)
    x_p = nc.declare_dram_parameter("x", [C2, N], BF16, isOutput=False)
    wpk_p = nc.declare_dram_parameter("wpk", [C2, PK_COLS], BF16, isOutput=False)
    out_p = nc.declare_dram_parameter("out", [C, N], F32, isOutput=True)

    with tile.TileContext(nc) as tc:
        with (
            tc.tile_pool(name="sb", bufs=1) as sb,
            tc.tile_pool(name="ps_s", bufs=3, space="PSUM") as ps_s,
            tc.tile_pool(name="ps_pv", bufs=2, space="PSUM") as ps_pv,
            tc.tile_pool(name="pp", bufs=8) as pp,
            tc.tile_pool(name="ep", bufs=2) as ep,
            tc.tile_pool(name="dram", bufs=2, space="DRAM") as dram,
        ):
            # ---- persistent SBUF tiles
            x_sb = sb.tile([C2, N], BF16, tag="x")
            wpk_sb = sb.tile([C2, PK_COLS], BF16, tag="wpk")
            wq_sb = wpk_sb[:, PK_WQ:PK_WQ + 128]
            wk_sb = wpk_sb[:, PK_WK:PK_WK + 128]
            wv_sb = wpk_sb[:, PK_WV:PK_WV + C]
            wp_sb = wpk_sb[:, PK_WP:PK_WP + C]
            fold_sb = wpk_sb[:, PK_FOLD:PK_FOLD + 256].bitcast(F32)
            g_sb = wpk_sb[:, PK_G:PK_G + 2].bitcast(F32)
            b_sb = wpk_sb[:, PK_B:PK_B + 2].bitcast(F32)
            eps_t = wpk_sb[:, PK_EPS:PK_EPS + 2].bitcast(F32)
            bc_sb = wpk_sb[:, PK_BC:PK_BC + 128]
            q_sb = sb.tile([C2, N], BF16, tag="q")    # head h rows 32h..32h+16
            k_sb = sb.tile([C2, N], BF16, tag="k")
            # per key-tile, per head: 32 cols = [16 V^T | 1 ones | 15 zeros]
            vt_sb = sb.tile([C2, MT * 128], F16, tag="vt")
            y_sb = sb.tile([C2, 2 * NC], F32, tag="y")  # proj out, fold layout
            yl_sb = sb.tile([C2, 2 * NC], F32, tag="yl")
            stats = sb.tile([C2, 4], F32, tag="stats")
            gwk = sb.tile([1, 2], F16, tag="gwk")   # gpsimd keep-awake scratch
            dmy_sb = sb.tile([8, 2], F32, tag="dmy")
            scr = sb.tile([C2, NC], BF16, tag="scr")  # HAM warm-up operand

            # ---- HAM warm-up: back-to-back dummy matmuls during the input
            # DMA window. ~4us of sustained PE busy flips the clock gate from
            # 4/8 (1.2 GHz) to 8/8 (2.4 GHz); the small inter-MM gaps of the
            # attention body then keep it warm. Nothing consumes the output.
            nc.vector.memset(scr[:], 0.0)
            for w in range(N_WARM):
                wp_ps = ps_pv.tile([C2, NC], F32, tag="pv")
                nc.tensor.matmul(wp_ps[:], lhsT=scr[:, 0:128], rhs=scr[:])

            # ---- prologue loads: x in 512-col pieces so the first QKV
            # matmul starts after 128KB; all small params in ONE packed DMA
            nc.sync.dma_start(x_sb[:, 0:512], x_p[:, 0:512])
            nc.scalar.dma_start(wpk_sb[:], wpk_p[:])
            nc.sync.dma_start(x_sb[:, 512:1024], x_p[:, 512:1024])
            nc.scalar.dma_start(x_sb[:, 1024:1536], x_p[:, 1024:1536])
            nc.sync.dma_start(x_sb[:, 1536:N], x_p[:, 1536:N])

            # V^T zero fill + ones columns on gpsimd (before it blocks on the
            # warm-up collective)
            nc.gpsimd.memset(vt_sb[:], 0.0)
            ones_ap = vt_sb[:].rearrange(
                "q (p h e) -> q p h e", p=MT, h=H, e=32)[:, :, :, 16:17]
            nc.gpsimd.memset(ones_ap, 1.0)

            # ---- QKV projections. q/k evacuated with head h at rows
            # 32h..32h+16 (stationary has zeros elsewhere).
            for c4 in range(4):
                cs = slice(512 * c4, 512 * (c4 + 1))
                qp = ps_pv.tile([C2, NC], F32, tag="pv")
                nc.tensor.matmul(qp[:], lhsT=wq_sb, rhs=x_sb[:, cs])
                nc.scalar.activation(q_sb[:, cs], qp[:], Act.Copy)
                kp = ps_pv.tile([C2, NC], F32, tag="pv")
                nc.tensor.matmul(kp[:], lhsT=wk_sb, rhs=x_sb[:, cs])
                nc.vector.tensor_copy(k_sb[:, cs], kp[:])

            # warm-up AllReduce: wakes the CC stream early so the real one at
            # the tail skips the cold-start latency.
            dm_in = dram.tile([8, 2], F32, tag="dm_in")
            dm_out = dram.tile([8, 2], F32, tag="dm_out")
            nc.gpsimd.memset(dmy_sb[:], 1.0)
            nc.gpsimd.dma_start(dm_in[:], dmy_sb[:])
            nc.gpsimd.collective_compute(
                "AllReduce", Alu.add,
                replica_groups=[list(range(N_CORES))],
                ins=[dm_in.opt()], outs=[dm_out.opt()])

            # all 16 V^T key tiles in ONE psum allocation
            vp_all = ps_s.tile([C2, MT * C], F32, tag="s")
            for p in range(MT):
                nc.tensor.matmul(vp_all[:, C * p:C * (p + 1)],
                                 lhsT=x_sb[:, 128 * p:128 * (p + 1)],
                                 rhs=wv_sb)
            vt_dst = vt_sb[:].rearrange(
                "q (p h e) -> q p h e", p=MT, h=H, e=32)[:, :, :, 0:16]
            vt_src = vp_all[:].rearrange(
                "q (p h d) -> q p h d", p=MT, h=H, d=D)
            nc.vector.tensor_copy(vt_dst, vt_src)

            def epilogue_pre(c, pv, last=False):
                """Denominator chain for chunk c: ln/exp + partition-broadcast.
                Mid-kernel chunks bounce the reciprocal rows through DRAM (the
                round-trip hides behind compute). The LAST chunk sits on the
                critical tail, so it broadcasts via one matmul against a
                row-selector matrix instead (saves the ~8us DMA round trip)."""
                if last:
                    # evacuate pv, broadcast the (positive) denominator rows
                    # to every partition via the selector matmul, then take
                    # the reciprocal of the broadcast result — NaN-free, and
                    # the mul's operands both end up in SBUF
                    pvs = ep.tile([C2, NC], BF16, tag="pvs")
                    nc.scalar.activation(pvs[:], pv[:], Act.Copy)
                    nc.gpsimd.tensor_copy(gwk[:], pvs[0:1, 0:2])
                    rbc_d = ps_s.tile([C2, 2 * NC], F32, tag="s")
                    nc.tensor.matmul(rbc_d[:, 0:NC], lhsT=bc_sb, rhs=pvs[:])
                    dln2 = ep.tile([C2, NC], F32, tag="dln")
                    nc.scalar.activation(dln2[:], rbc_d[:, 0:NC], Act.Ln)
                    rbc = ep.tile([C2, NC], F32, tag="rbc")
                    nc.scalar.activation(rbc[:], dln2[:], Act.Exp, scale=-1.0)
                    nc.gpsimd.tensor_copy(gwk[:], rbc[0:1, 0:2])
                    return (pvs, rbc)
                dln = ep.tile([C2, NC], F32, tag="dln")
                nc.scalar.activation(dln[:], pv[:], Act.Ln)
                drc = ep.tile([C2, NC], F32, tag="drc")
                nc.scalar.activation(drc[:], dln[:], Act.Exp, scale=-1.0)
                rbc = ep.tile([C2, NC], F32, tag="rbc")
                rec_d = dram.tile([H, NC], F32, tag="rec_d")
                for h in range(H):
                    nc.sync.dma_start(rec_d[h:h + 1, :],
                                      drc[32 * h + 16:32 * h + 17, :])
                for h in range(H):
                    nc.sync.dma_start(
                        rbc[32 * h:32 * h + 32, :],
                        rec_d[h:h + 1, :].partition_broadcast(32))
                return rbc

            def epilogue_post(c, pv, rbc, last=False):
                """Normalize chunk-c attention output, project, evac + stats.
                The proj-output PSUM tile comes from the pv ring (the slot the
                normalize just freed) so the scores ring keeps all 3 slots."""
                on = ep.tile([C2, NC], BF16, tag="on")
                if last:
                    pvs, rbc_sb = rbc
                    nc.vector.tensor_mul(on[:], pvs[:], rbc_sb[:])
                    nc.gpsimd.tensor_copy(gwk[:], on[0:1, 0:2])
                else:
                    nc.vector.tensor_mul(on[:], pv[:], rbc[:])
                yp = ps_s.tile([C2, 2 * NC], F32, tag="s")
                r = slice(64 * (c // 2), 64 * (c // 2) + 64)
                nc.tensor.matmul(yp[r, 0:NC], lhsT=wp_sb, rhs=on[:],
                                 tile_position=(0, 64 * (c // 2)))
                ycols = slice(512 * (c % 2), 512 * (c % 2) + 512)
                s0 = 2 * (c % 2)
                nc.vector.tensor_scalar(y_sb[r, ycols], yp[r, 0:NC], 1.0, 0.0,
                                        op0=Alu.mult, op1=Alu.add,
                                        accum_out=stats[r, s0:s0 + 1])
                ysq = ep.tile([C2, NC], F32, tag="dln")
                if last:
                    # critical tail: sumsq on the scalar engine in parallel
                    # with the DVE evac (both read yp from PSUM)
                    nc.scalar.activation(ysq[r, :], yp[r, 0:NC], Act.Square,
                                         accum_out=stats[r, s0 + 1:s0 + 2])
                else:
                    nc.vector.scalar_tensor_tensor(
                        ysq[r, :], y_sb[r, ycols], 0.0, y_sb[r, ycols],
                        op0=Alu.add, op1=Alu.mult,
                        accum_out=stats[r, s0 + 1:s0 + 2])

            # stats-reduce staging (split: top chunk-pair mid-kernel, bottom
            # pair at the tail). AllReduce on [C,4] — roughly half the CC
            # latency of the AllGather-and-add-locally scheme.
            rb_a = sb.tile([C2, 4], F32, tag="rb_a")
            rb_b = sb.tile([C2, 4], F32, tag="rb_b")
            fa = sb.tile([C2, 2], F32, tag="fa")
            st_in_a = dram.tile([C, 4], F32, tag="st_in_a")
            st_out_a = dram.tile([C, 4], F32, tag="st_out_a")
            st_in_b = dram.tile([C, 4], F32, tag="st_in_b")
            st_out_b = dram.tile([C, 4], F32, tag="st_out_b")

            # ---- attention: per (chunk, key-tile): 4 concurrent row-tiled
            # score MMs; exp split scalar/vector; 4 concurrent col-tiled PV
            # MMs lagged one tile so the PE never waits on exp. The loop is
            # FLAT over all 64 (chunk, tile) pairs so the next chunk's score
            # MMs interleave with the previous chunk's trailing PV flushes —
            # no pipeline drain at chunk boundaries.
            prev = None
            prev_rbc = None
            pends = []
            pv = None

            def flush():
                """PV block for the OLDEST pending tile (its own chunk's
                accumulator rides along in the entry). Two-tile lag: by the
                time these hit the PE queue head, both exp paths have long
                finished, so the in-order queue never stalls."""
                if not pends:
                    return
                p01, p23, t, pvx = pends.pop(0)
                for h in range(H):
                    src = p01 if h < 2 else p23
                    nc.tensor.matmul(
                        pvx[32 * h:32 * h + 32, :],
                        lhsT=vt_sb[:, 128 * t + 32 * h:128 * t + 32 * h + 32],
                        rhs=src[:, 512 * (h % 2):512 * (h % 2) + 512],
                        start=(t == 0), stop=(t == MT - 1),
                        skip_group_check=True,
                        tile_position=(0, 32 * h))

            for T in range(NCH * MT):
                c, t = divmod(T, MT)
                qs = slice(NC * c, NC * (c + 1))
                if t == 0:
                    if pv is not None:
                        prev = (c - 1, pv)
                    pv = ps_pv.tile([C2, NC], F32, tag="pv")
                if True:
                    # t==2, not earlier: the previous chunk's tile-15 PV
                    # flush is emitted during t==1, and the Ln must follow it
                    # in program order (Tile would otherwise order that PV
                    # write after this read)
                    if c > 0 and t == 2 and prev is not None:
                        prev_rbc = epilogue_pre(*prev)
                    if c > 0 and t == 12 and prev is not None:
                        epilogue_post(*prev, prev_rbc)
                        prev = None
                        if c == 2:
                            # chunks 0,1 stats (rows 0:64) are final: reduce
                            # them now, hidden under chunks 2,3 compute
                            nc.sync.dma_start(st_in_a[:], stats[0:C, 0:4])
                            nc.gpsimd.collective_compute(
                                "AllReduce", Alu.add,
                                replica_groups=[list(range(N_CORES))],
                                ins=[st_in_a.opt()], outs=[st_out_a.opt()])
                            nc.sync.dma_start(rb_a[0:C, :], st_out_a[:])
                            nc.gpsimd.tensor_add(fa[0:C, :], rb_a[0:C, 0:2],
                                                 rb_a[0:C, 2:4])
                        if c == 3:
                            # chunk 2's half of the bottom stats can ship
                            # ahead of the tail reduce
                            nc.sync.dma_start(st_in_b[:, 0:2],
                                              stats[C:C2, 0:2])
                    # pool rotation note: with 3 slots and 2 tiles/t, the
                    # sp01-first order gives DVE-freed slots two tiles of
                    # slack and scalar-freed slots one — measured fastest.
                    sp01 = ps_s.tile([C2, 1024], F32, tag="s")
                    sp23 = ps_s.tile([C2, 1024], F32, tag="s")
                    for h in range(H):
                        sp = sp01 if h < 2 else sp23
                        nc.tensor.matmul(
                            sp[:, 512 * (h % 2):512 * (h % 2) + 512],
                            lhsT=k_sb[32 * h:32 * h + 16, 128 * t:128 * (t + 1)],
                            rhs=q_sb[32 * h:32 * h + 16, qs],
                            tile_position=(32 * h, 0))
                    # exp now (other engines), PV one tile behind
                    p01 = pp.tile([C2, 1024], F16, tag="p")
                    nc.scalar.activation(p01[:], sp01[:], Act.Exp,
                                         scale=1.0 / A_EXP)
                    p23 = pp.tile([C2, 1024], F16, tag="p")
                    nc.vector.tensor_scalar(p23[:].bitcast(I16), sp23[:],
                                            1.0, B_EXP,
                                            op0=Alu.mult, op1=Alu.add)
                    if c == NCH - 1 and t >= 10:
                        # keep gpsimd awake near the tail so the collective
                        # trigger doesn't pay its multi-us idle-exit latency
                        nc.gpsimd.tensor_copy(gwk[:], p23[0:1, 0:2])
                    if len(pends) == 2:
                        flush()
                    pends.append((p01, p23, t, pv))
            flush()
            flush()
            prev = (NCH - 1, pv)
            prev_rbc = epilogue_pre(*prev, last=True)
            epilogue_post(*prev, prev_rbc, last=True)

            # ---- reduce the bottom chunk-pair stats (rows 64:128); chunk 2's
            # half shipped during chunk 3, only chunk 3's 256B go here
            nc.sync.dma_start(st_in_b[:, 2:4], stats[C:C2, 2:4])
            nc.gpsimd.collective_compute(
                "AllReduce", Alu.add,
                replica_groups=[list(range(N_CORES))],
                ins=[st_in_b.opt()], outs=[st_out_b.opt()])
            nc.sync.dma_start(rb_b[C:C2, :], st_out_b[:])
            nc.vector.tensor_add(fa[C:C2, :], rb_b[C:C2, 0:2],
                                 rb_b[C:C2, 2:4])
            # fold the channel halves + replicate to 128 partitions in one
            # matmul against the pair-identity matrix
            fps = ps_pv.tile([C2, NC], F32, tag="pv")
            nc.tensor.matmul(fps[:, 0:2], lhsT=fold_sb, rhs=fa[:])

            # ---- finalize: mean/var -> scale/shift. The variance pieces and
            # -gamma*mean run on the (idle) DVE in parallel with the scalar
            # chain, cutting the serial depth to ms->lnv->istd->{sh,sc}.
            ms = sb.tile([C2, 2], F32, tag="ms")
            nc.scalar.activation(ms[:], fps[:, 0:2], Act.Copy, scale=1.0 / CNT)
            msq = sb.tile([C2, 1], F32, tag="msq")
            nc.vector.tensor_mul(msq[:], ms[:, 0:1], ms[:, 0:1])
            var = sb.tile([C2, 1], F32, tag="var")
            nc.vector.scalar_tensor_tensor(var[:], msq[:], -1.0, ms[:, 1:2],
                                           op0=Alu.mult, op1=Alu.add)
            ngm = sb.tile([C2, 1], F32, tag="ngm")
            nc.vector.scalar_tensor_tensor(ngm[:], g_sb[:, 0:1], -1.0,
                                           ms[:, 0:1],
                                           op0=Alu.mult, op1=Alu.mult)
            lnv = sb.tile([C2, 1], F32, tag="lnv")
            nc.scalar.activation(lnv[:], var[:], Act.Ln, bias=eps_t[:, 0:1])
            istd = sb.tile([C2, 1], F32, tag="istd")
            nc.scalar.activation(istd[:], lnv[:], Act.Exp, scale=-0.5)
            sh = sb.tile([C2, 1], F32, tag="sh")
            nc.scalar.activation(sh[:], ngm[:], Act.Identity,
                                 scale=istd[:, 0:1], bias=b_sb[:, 0:1])
            sc = sb.tile([C2, 1], F32, tag="sc")
            nc.scalar.activation(sc[:], g_sb[:, 0:1], Act.Copy,
                                 scale=istd[:, 0:1])

            # ---- BN scale/shift + LeakyReLU as a split Prelu so the first
            # output DMA overlaps the second half's compute
            nc.scalar.activation(yl_sb[:, 0:NC], y_sb[:, 0:NC], Act.Prelu,
                                 scale=sc[:, 0:1], bias=sh[:, 0:1], alpha=LEAK)
            nc.sync.dma_start(out_p[:, 0:512], yl_sb[0:C, 0:NC])
            nc.gpsimd.dma_start(out_p[:, 1024:1536], yl_sb[C:C2, 0:NC])
            nc.scalar.activation(yl_sb[:, NC:2 * NC], y_sb[:, NC:2 * NC],
                                 Act.Prelu,
                                 scale=sc[:, 0:1], bias=sh[:, 0:1], alpha=LEAK)
            nc.sync.dma_start(out_p[:, 512:1024], yl_sb[0:C, NC:2 * NC])
            nc.scalar.dma_start(out_p[:, 1536:N], yl_sb[C:C2, NC:2 * NC])

    nc.compile()

    # Post-compile surgery: one activation table set covers every function
    # used here (Exp, Ln, Copy, Prelu); point the first load at it and drop
    # the rest so the table-load inserter doesn't ping-pong.
    from concourse.hw_specs import get_activation_tables
    tabs = list(get_activation_tables(nc.m.arch).keys())
    nle = tabs.index("natural_log_exp_and_others")
    loads = [(b, i) for b in nc.main_func.blocks for i in b.instructions
             if isinstance(i, mybir.InstLoadActFuncSet)]
    if loads:
        loads[0][1].act_func_set_id = nle
        for b, i in loads[1:]:
            b.instructions.remove(i)
    return nc


_NC_CACHE = None


def _get_nc():
    global _NC_CACHE
    if _NC_CACHE is None:
        _NC_CACHE = build()
    return _NC_CACHE


def _prep_inputs(x_local, x_branch, w_qkv, w_proj, gamma, beta):
    bf16 = ml_dtypes.bfloat16
    x_local = np.asarray(x_local, np.float32)
    x_branch = np.asarray(x_branch, np.float32)
    w_qkv = np.asarray(w_qkv, np.float32)
    w_proj = np.asarray(w_proj, np.float32)
    gamma = np.asarray(gamma, np.float32)
    beta = np.asarray(beta, np.float32)

    X = np.concatenate([x_local, x_branch], axis=1).astype(bf16)  # [B, 128, N]
    WT = w_qkv.T.copy()  # [128, 192]
    wq = np.zeros((C2, 128), np.float32)
    wk = np.zeros((C2, 128), np.float32)
    for h in range(H):
        wq[:, 32 * h:32 * h + D] = WT[:, D * h:D * (h + 1)]
        # fold the softmax scale and the Schraudolph slope into wk
        wk[:, 32 * h:32 * h + D] = WT[:, C + D * h:C + D * (h + 1)] * (A_EXP * SCALE)
    wv = WT[:, 2 * C:3 * C]
    wp = np.zeros((C2, C), np.float32)
    for h in range(H):
        wp[32 * h:32 * h + D, :] = w_proj[:, D * h:D * (h + 1)].T
    fold = np.zeros((C2, 128), np.float32)
    for r in range(C2):
        fold[r, r % 64] = 1.0
        fold[r, r % 64 + 64] = 1.0

    # denom-broadcast selector: out[m, q] = drc[32*(m//32)+16, q]
    bc = np.zeros((C2, 128), np.float32)
    for m in range(128):
        bc[32 * (m // 32) + 16, m] = 1.0

    # packed param blob: bf16 matrices + f32 regions viewed as bf16 pairs
    wpk = np.zeros((C2, PK_COLS), bf16)
    wpk[:, PK_WQ:PK_WQ + 128] = wq.astype(bf16)
    wpk[:, PK_WK:PK_WK + 128] = wk.astype(bf16)
    wpk[:, PK_WV:PK_WV + C] = np.ascontiguousarray(wv).astype(bf16)
    wpk[:, PK_WP:PK_WP + C] = wp.astype(bf16)
    wpk[:, PK_BC:PK_BC + 128] = bc.astype(bf16)

    def pack_f32(col, arr):
        raw = np.ascontiguousarray(arr, np.float32)
        wpk[:, col:col + 2 * raw.shape[1]] = raw.view(np.uint16).view(bf16)

    pack_f32(PK_FOLD, fold)
    g2 = np.concatenate([gamma, gamma]).reshape(C2, 1)
    b2 = np.concatenate([beta, beta]).reshape(C2, 1)
    pack_f32(PK_G, g2)
    pack_f32(PK_B, b2)
    pack_f32(PK_EPS, np.full((C2, 1), BN_EPS, np.float32))

    return [dict(x=np.ascontiguousarray(X[b]), wpk=wpk) for b in range(B)]


def kernel(x_local, x_branch, w_qkv, w_proj, gamma, beta, _trace=False, _tmpdir=None):
    nc = _get_nc()
    in_maps = _prep_inputs(x_local, x_branch, w_qkv, w_proj, gamma, beta)
    res = run_bass_kernel_spmd(nc, in_maps, core_ids=list(range(N_CORES)),
                               trace=_trace, tmpdir=_tmpdir)
    out = np.stack([np.asarray(res.results[i]["out"]) for i in range(N_CORES)])
    if _trace:
        kernel._last_results = res
    return out.astype(np.float32)
